# revision 62
# baseline (speedup 1.0000x reference)
"""Bass kernel builder for nn_MixtureOfMambaBlock — 8-core SPMD, v2.

Sharding: tokens 8-way, fully local (no collectives). Each core owns 512
contiguous tokens (+128-token halo for conv + scan warmup; the scan decay
(1-dt)~0.5 makes 128 steps numerically exact). The MoE is evaluated densely
for all 4 experts on the core's own 512 tokens (same FLOPs as expert-
parallel, zero communication). Weights replicated, bf16 matmuls, weight
layouts pre-swizzled on host for contiguous DMA.
"""
import numpy as np
import concourse.bass as bass
import concourse.bacc as bacc
import concourse.mybir as mybir
import concourse.tile as tile

FP = mybir.dt.float32
FR = mybir.dt.float32r
BF = mybir.dt.bfloat16
F8 = mybir.dt.float8e4
DR = mybir.MatmulPerfMode.DoubleRow
AF = mybir.ActivationFunctionType
ALU = mybir.AluOpType
ESCALE = 32.0            # fp8 expert-weight scale

B, T, D = 2, 2048, 1024
S, INNER = 64, 2048
E, HID = 4, 4096
OWN, HALO = 512, 128
NH = OWN + HALO          # 640
KB = D // 128            # 8  d-blocks
MB = INNER // 128        # 16 inner-blocks
OTB = OWN // 128         # 4  own-token blocks
HB = HID // 128          # 32 hid-blocks
N_CORES = 8

INPUT_SPECS = {
    "x_sh": ([NH, D], FP),
    "ipw": ([D, 2 * INNER], BF),       # norm1 folded
    "ipb": ([128, 2 * INNER // 128], FP),
    "cw": ([128, MB, 3], FP), "cb": ([128, MB], FP),
    "dtw": ([128, MB, S], BF), "dtb": ([S, 1], FP),
    "bpw": ([128, MB, S], BF), "bpb": ([S, 1], FP),
    "cpw": ([128, MB, S], BF), "cpb": ([S, 1], FP),
    "s2iw": ([S, INNER], BF), "s2ib": ([128, MB], FP),
    "Dp": ([128, MB], FP),
    "ow": ([INNER, D], BF), "ob": ([1, D], BF),
    "gw": ([128, KB, E], FP), "gb": ([1, E], FP),   # norm2 folded into gw
    "ew1": ([E * D, HID], BF),         # norm2 folded; rows e*1024+d
    "eb1": ([128, E * HB], FP),
    "ew2": ([E * HID // 2, 2 * D], F8),  # x32; row (e,hp,p) = [p, two, d]
    "eb2": ([1, E * D], F8),           # x32
    "ident": ([128, 128], FP),
    "identb": ([128, 128], BF),
    "ones1": ([1, 128], BF),
    "ones18": ([1, 128], F8),
    "ones1f": ([1, 128], FP),
}


def build(eb2_zero=True):
    nc = bacc.Bacc("TRN2", target_bir_lowering=False, debug=False,
                   num_devices=N_CORES)
    dp = {}
    for name, (shape, dt) in INPUT_SPECS.items():
        dp[name] = nc.dram_tensor(name, shape, dt, kind="ExternalInput")
    out_d = nc.dram_tensor("out", [OWN, D], FP, kind="ExternalOutput")

    with tile.TileContext(nc) as tc:
        with tc.tile_pool(name="outer", bufs=1) as po:
            # ident first (the rms1 transposes hang on it), then x
            ident = po.tile([128, 128], FP)
            nc.sync.dma_start(ident[:], dp["ident"][:])
            identb = po.tile([128, 128], BF)
            nc.sync.dma_start(identb[:], dp["identb"][:])
            xh = po.tile([128, D], FP, name="xh")
            nc.sync.dma_start(xh[:], dp["x_sh"][0:128, :])
            xo = [po.tile([128, D], FP, name=f"xo{t_}", tag=f"xo{t_}")
                  for t_ in range(OTB)]
            for t_ in range(OTB):
                nc.sync.dma_start(xo[t_][:],
                                  dp["x_sh"][(t_ + 1) * 128:(t_ + 2) * 128, :])
            # pre-warm ACT function tables during the DMA head (earliest-needed last)
            warm = po.tile([1, 1], FP, name="warm")
            for fn in (AF.Gelu, AF.Sigmoid, AF.Sqrt, AF.Square):
                nc.scalar.activation(warm[:], ident[0:1, 0:1], fn)
            # ---------- constants / persistent ----------
            ones1 = po.tile([1, 128], BF)
            nc.sync.dma_start(ones1[:], dp["ones1"][:])
            ones1f = po.tile([1, 128], FP)
            nc.sync.dma_start(ones1f[:], dp["ones1f"][:])
            ob_sb = po.tile([1, D], BF)
            nc.sync.dma_start(ob_sb[:], dp["ob"][:])
            gb_sb = po.tile([1, E], FP)
            nc.sync.dma_start(gb_sb[:], dp["gb"][:])
            if not eb2_zero:
                ones18 = po.tile([1, 128], F8)
                nc.sync.dma_start(ones18[:], dp["ones18"][:])
                eb2_sb = po.tile([1, E * D], F8)
                nc.sync.dma_start(eb2_sb[:], dp["eb2"][:])
            eb1_sb = po.tile([128, E * HB], FP)
            nc.sync.dma_start(eb1_sb[:], dp["eb1"][:])
            gw_sb = po.tile([128, KB, E], FP)
            nc.sync.dma_start(gw_sb[:], dp["gw"][:])

            # persistent activations
            xmid = [po.tile([128, D], FP, name=f"xmid{t_}", tag=f"xmid{t_}")
                    for t_ in range(OTB)]
            h2T = po.tile([128, KB, OWN], BF, name="h2T")
            wv = [po.tile([128, E], FP, name=f"wv{t_}", tag=f"wv{t_}")
                  for t_ in range(OTB)]
            # ew1 streamed in eighths (512 hid cols), double-buffered
            NQ = 8          # eighths per expert
            QW = HID // NQ  # 512

            def load_ew1q(g):  # g = e*NQ + q, into tag g%2
                e, q = divmod(g, NQ)
                t = po.tile([128, KB, QW], BF, tag=f"ew1q{g % 2}",
                            name=f"ew1q{g % 2}")
                for kb in range(KB):
                    nc.sync.dma_start(
                        t[:, kb, :],
                        dp["ew1"][e * D + kb * 128: e * D + (kb + 1) * 128,
                                  q * QW:(q + 1) * QW])
                return t

            # =======================================================
            # MIXER
            # =======================================================
            with (
                tc.tile_pool(name="mixer", bufs=1) as pm,
                tc.tile_pool(name="mixt", bufs=1) as pt_pool,
            ):
                hT = [pm.tile([128, NH], BF, name=f"hT{kb}", tag=f"hT{kb}")
                      for kb in range(KB)]
                xm = [pm.tile([128, NH], BF, name=f"xm{m}", tag=f"xm{m}")
                      for m in range(MB)]

                # ---- rmsnorm1 + transpose to hT (bf16) ----
                with nc.named_scope("rms1"), tc.tile_pool(name="ps1", bufs=1, space="PSUM") as psA:
                    for tb in range(NH // 128):
                        xt = xh if tb == 0 else xo[tb - 1]
                        scr = pt_pool.tile([128, D], FP, tag="scr", bufs=2)
                        sq = pt_pool.tile([128, 1], FP, tag="sq", bufs=2)
                        nc.scalar.activation(scr[:], xt[:], AF.Square, accum_out=sq[:])
                        nr = pt_pool.tile([128, 1], FP, tag="nr", bufs=2)
                        nc.vector.tensor_scalar(nr[:], sq[:], 1.0 / D, 1e-6, ALU.mult, ALU.add)
                        nc.scalar.sqrt(nr[:], nr[:])
                        nc.vector.reciprocal(nr[:], nr[:])
                        h_t = pt_pool.tile([128, D], BF, tag="htb", bufs=2)
                        nc.vector.tensor_scalar(h_t[:], xt[:], nr[:], None, ALU.mult)
                        for kb in range(KB):
                            ptr = psA.tile([128, 128], BF, tag="ptr", bufs=2)
                            nc.tensor.transpose(ptr[:], h_t[:, kb * 128:(kb + 1) * 128],
                                                identb[:])
                            nc.vector.tensor_copy(hT[kb][:, tb * 128:(tb + 1) * 128], ptr[:])

                ipb_sb = pm.tile([128, 2 * INNER // 128], FP)
                nc.sync.dma_start(ipb_sb[:], dp["ipb"][:])
                cb_sb = pm.tile([128, MB], FP)
                nc.sync.dma_start(cb_sb[:], dp["cb"][:])
                cw_sb = pm.tile([128, MB, 3], FP)
                nc.sync.dma_start(cw_sb[:], dp["cw"][:])

                # ---- in_proj (x_main half) + conv + silu ----
                with nc.named_scope("in_proj"), tc.tile_pool(name="ps2", bufs=1, space="PSUM") as psA:
                    for q in range(4):
                        wq = []
                        for kb in range(KB):
                            wt = pt_pool.tile([128, 512], BF, tag=f"wip{kb}", bufs=2,
                                              name=f"wip{kb}")
                            nc.gpsimd.dma_start(
                                wt[:], dp["ipw"][kb * 128:(kb + 1) * 128,
                                                 q * 512:(q + 1) * 512])
                            wq.append(wt)
                        for mi in range(4):
                            m = q * 4 + mi
                            xzp = pt_pool.tile([128, NH + 2], FP, tag="xzp", bufs=2)
                            nc.vector.memset(xzp[:, 0:2], 0.0)
                            for n0, nw in ((0, 512), (512, 128)):
                                px = psA.tile([128, 512], FP, tag="px", bufs=3)
                                for kb in range(KB):
                                    nc.tensor.matmul(px[:, 0:nw],
                                                     wq[kb][:, mi * 128:(mi + 1) * 128],
                                                     hT[kb][:, n0:n0 + nw],
                                                     start=(kb == 0), stop=(kb == KB - 1))
                                nc.scalar.activation(xzp[:, 2 + n0:2 + n0 + nw], px[:, 0:nw],
                                                     AF.Identity, bias=ipb_sb[:, m:m + 1])
                            cv = pt_pool.tile([128, NH], FP, tag="cv", bufs=2)
                            nc.vector.tensor_scalar(cv[:], xzp[:, 0:NH], cw_sb[:, m, 0:1],
                                                    None, ALU.mult)
                            nc.vector.scalar_tensor_tensor(cv[:], xzp[:, 1:1 + NH],
                                                           cw_sb[:, m, 1:2], cv[:],
                                                           ALU.mult, ALU.add)
                            nc.vector.scalar_tensor_tensor(cv[:], xzp[:, 2:2 + NH],
                                                           cw_sb[:, m, 2:3], cv[:],
                                                           ALU.mult, ALU.add)
                            sgc = pt_pool.tile([128, NH], FP, tag="sgc", bufs=2)
                            nc.scalar.activation(sgc[:], cv[:], AF.Sigmoid, bias=cb_sb[:, m:m + 1])
                            nc.vector.scalar_tensor_tensor(xm[m][:], cv[:], cb_sb[:, m:m + 1],
                                                           sgc[:], ALU.add, ALU.mult)

                dtb_sb = pm.tile([S, 1], FP)
                nc.sync.dma_start(dtb_sb[:], dp["dtb"][:])
                bpb_sb = pm.tile([S, 1], FP)
                nc.sync.dma_start(bpb_sb[:], dp["bpb"][:])
                cpb_sb = pm.tile([S, 1], FP)
                nc.sync.dma_start(cpb_sb[:], dp["cpb"][:])
                dtw_sb = pm.tile([128, MB, S], BF)
                nc.sync.dma_start(dtw_sb[:], dp["dtw"][:])
                bpw_sb = pm.tile([128, MB, S], BF)
                nc.sync.dma_start(bpw_sb[:], dp["bpw"][:])
                cpw_sb = pm.tile([128, MB, S], BF)
                nc.sync.dma_start(cpw_sb[:], dp["cpw"][:])
                ew1q = load_ew1q(0)

                # ---- dt/B/C projections + scan ----
                with nc.named_scope("scan"), tc.tile_pool(name="ps3", bufs=1, space="PSUM") as psA:
                    dt_t = pt_pool.tile([S, NH], FP, tag="dt")
                    a_t = pt_pool.tile([S, NH], FP, tag="a")
                    b_t = pt_pool.tile([S, NH], FP, tag="b")
                    c_t = pt_pool.tile([S, NH], FP, tag="c")
                    for n0, nw in ((0, 512), (512, 128)):
                        for wsb, bias_sb, dst, fn in (
                            (dtw_sb, dtb_sb, dt_t, AF.Sigmoid),
                            (cpw_sb, cpb_sb, c_t, AF.Identity),
                        ):
                            pz = psA.tile([S, 512], FP, tag="pz", bufs=2)
                            for kb in range(MB):
                                nc.tensor.matmul(pz[:, 0:nw], wsb[:, kb, :],
                                                 xm[kb][:, n0:n0 + nw],
                                                 start=(kb == 0), stop=(kb == MB - 1))
                            nc.scalar.activation(dst[:, n0:n0 + nw], pz[:, 0:nw], fn,
                                                 bias=bias_sb[:])
                        # b needs dt -> separate pass
                        pz = psA.tile([S, 512], FP, tag="pz", bufs=2)
                        for kb in range(MB):
                            nc.tensor.matmul(pz[:, 0:nw], bpw_sb[:, kb, :],
                                             xm[kb][:, n0:n0 + nw],
                                             start=(kb == 0), stop=(kb == MB - 1))
                        nc.vector.scalar_tensor_tensor(b_t[:, n0:n0 + nw], pz[:, 0:nw],
                                                       bpb_sb[:], dt_t[:, n0:n0 + nw],
                                                       ALU.add, ALU.mult)
                    nc.scalar.activation(a_t[:], dt_t[:], AF.Identity, bias=1.0, scale=-1.0)
                    st_t = pt_pool.tile([S, NH], FP, tag="st")
                    nc.vector.tensor_tensor_scan(st_t[:], a_t[:], b_t[:], 0.0,
                                                 ALU.mult, ALU.add)
                    y_t = pt_pool.tile([S, OWN], FP, tag="dt", name="y_t")
                    nc.vector.tensor_mul(y_t[:], c_t[:, HALO:NH], st_t[:, HALO:NH])

                # ---- layernorm over S (transpose - LN - transpose back) ----
                s2ib_sb = pm.tile([128, MB], FP)
                nc.sync.dma_start(s2ib_sb[:], dp["s2ib"][:])
                Dp_sb = pm.tile([128, MB], FP)
                nc.sync.dma_start(Dp_sb[:], dp["Dp"][:])
                s2iw_sb = pm.tile([S, INNER], BF)
                nc.sync.dma_start(s2iw_sb[:], dp["s2iw"][:])

                # ---- LN over S + s2i + gate sigmoid + pre_out assembly ----
                # gate-half (pg) matmuls are independent of yln; interleave
                # their emission with the LN iterations so the in-order PE
                # queue has work while LN's serial DVE chain runs.
                with nc.named_scope("premix"), tc.tile_pool(name="ps5", bufs=1, space="PSUM") as psA:
                    pre = []
                    wqs = {}
                    pgs = {}

                    def emit_pg(m):
                        q, mi = divmod(m, 4)
                        if mi == 0:
                            wq = []
                            for kb in range(KB):
                                wt = pt_pool.tile([128, 512], BF, tag=f"wip{kb}", bufs=2,
                                                  name=f"wipg{kb}")
                                nc.gpsimd.dma_start(
                                    wt[:], dp["ipw"][kb * 128:(kb + 1) * 128,
                                                     2048 + q * 512:2048 + (q + 1) * 512])
                                wq.append(wt)
                            wqs[q] = wq
                        pg = psA.tile([128, 512], FP, tag="pg", bufs=4)
                        for kb in range(KB):
                            nc.tensor.matmul(pg[:], wqs[q][kb][:, mi * 128:(mi + 1) * 128],
                                             hT[kb][:, HALO:NH],
                                             start=(kb == 0), stop=(kb == KB - 1))
                        pgs[m] = pg

                    yln = pt_pool.tile([S, OWN], BF, tag="a", name="yln")
                    for i in range(OTB):
                        if i < 3:
                            emit_pg(i)
                        ptr = psA.tile([128, 128], FP, tag="ptr", bufs=1)
                        nc.tensor.transpose(ptr[:, 0:S], y_t[:, i * 128:(i + 1) * 128],
                                            ident[0:S, 0:S])
                        yT = pt_pool.tile([128, S], FP, tag="yT", bufs=2)
                        nc.vector.tensor_copy(yT[:], ptr[:, 0:S])
                        mu = pt_pool.tile([128, 1], FP, tag="mu", bufs=2)
                        nc.vector.tensor_reduce(mu[:], yT[:], mybir.AxisListType.X, ALU.add)
                        nc.vector.tensor_scalar_mul(mu[:], mu[:], 1.0 / S)
                        xc = pt_pool.tile([128, S], FP, tag="xc", bufs=2)
                        nc.vector.tensor_scalar_sub(xc[:], yT[:], mu[:])
                        scr2 = pt_pool.tile([128, S], FP, tag="scr2", bufs=2)
                        vv = pt_pool.tile([128, 1], FP, tag="vv", bufs=2)
                        nc.scalar.activation(scr2[:], xc[:], AF.Square, accum_out=vv[:])
                        nc.vector.tensor_scalar(vv[:], vv[:], 1.0 / S, 1e-5, ALU.mult, ALU.add)
                        nc.scalar.sqrt(vv[:], vv[:])
                        nc.vector.reciprocal(vv[:], vv[:])
                        nc.vector.tensor_scalar_mul(xc[:], xc[:], vv[:])
                        ptr2 = psA.tile([128, 128], FP, tag="ptr2", bufs=1)
                        nc.tensor.transpose(ptr2[0:S, :], xc[:], ident[:])
                        nc.vector.tensor_copy(yln[:, i * 128:(i + 1) * 128], ptr2[0:S, :])

                    for m in range(MB):
                        if m + 3 < MB:
                            emit_pg(m + 3)
                        ps = psA.tile([128, 512], FP, tag="ps", bufs=2)
                        nc.tensor.matmul(ps[:], s2iw_sb[:, m * 128:(m + 1) * 128], yln[:],
                                         start=True, stop=True)
                        sg = pt_pool.tile([128, OWN], FP, tag="sg", bufs=2)
                        nc.scalar.activation(sg[:], pgs.pop(m)[:], AF.Sigmoid,
                                             bias=ipb_sb[:, MB + m:MB + m + 1])
                        tmp = pt_pool.tile([128, OWN], FP, tag="tmp", bufs=2)
                        nc.vector.tensor_scalar(tmp[:], xm[m][:, HALO:NH],
                                                Dp_sb[:, m:m + 1], None, ALU.mult)
                        nc.vector.scalar_tensor_tensor(tmp[:], ps[:], s2ib_sb[:, m:m + 1],
                                                       tmp[:], ALU.add, ALU.add)
                        pre_m = pm.tile([128, OWN], BF, tag=f"xm{m}", name=f"pre{m}")
                        nc.vector.tensor_mul(pre_m[:], tmp[:], sg[:])
                        pre.append(pre_m)

                # ---- out projection + residual + rms2 + h2T + gating ----
                with nc.named_scope("outproj"), tc.tile_pool(name="ps6", bufs=1, space="PSUM") as psA:
                    for nb in range(2):
                        po_t = [psA.tile([128, 512], FP, tag=f"po{t_}", bufs=1,
                                         name=f"po{nb}_{t_}") for t_ in range(OTB)]
                        for kb in range(MB):
                            owt = pt_pool.tile([128, 512], BF, tag="owt", bufs=4)
                            nc.gpsimd.dma_start(
                                owt[:], dp["ow"][kb * 128:(kb + 1) * 128,
                                                 nb * 512:(nb + 1) * 512])
                            for t_ in range(OTB):
                                nc.tensor.matmul(po_t[t_][:],
                                                 pre[kb][:, t_ * 128:(t_ + 1) * 128],
                                                 owt[:], start=(kb == 0), stop=False)
                        for t_ in range(OTB):
                            nc.tensor.matmul(po_t[t_][:], ones1[:],
                                             ob_sb[:, nb * 512:(nb + 1) * 512],
                                             start=False, stop=True)
                            nc.vector.tensor_add(xmid[t_][:, nb * 512:(nb + 1) * 512],
                                                 po_t[t_][:],
                                                 xo[t_][:, nb * 512:(nb + 1) * 512])
                    for tb in range(OTB):
                        # rms2 for this tb
                        scr = pt_pool.tile([128, D], FP, tag="scr", bufs=2)
                        sq = pt_pool.tile([128, 1], FP, tag="sq", bufs=2)
                        nc.scalar.activation(scr[:], xmid[tb][:], AF.Square, accum_out=sq[:])
                        nr = pt_pool.tile([128, 1], FP, tag="nr", bufs=2)
                        nc.vector.tensor_scalar(nr[:], sq[:], 1.0 / D, 1e-6, ALU.mult, ALU.add)
                        nc.scalar.sqrt(nr[:], nr[:])
                        nc.vector.reciprocal(nr[:], nr[:])
                        h2 = pt_pool.tile([128, D], FP, tag="xt", bufs=2, name="h2")
                        nc.vector.tensor_scalar(h2[:], xmid[tb][:], nr[:], None, ALU.mult)
                        pl = psA.tile([128, E], FP, tag="pl", bufs=2)
                        for kb in range(KB):
                            ptr = psA.tile([128, 128], FP, tag="ptr", bufs=2)
                            nc.tensor.transpose(ptr[:], h2[:, kb * 128:(kb + 1) * 128], ident[:])
                            h2T_t = pt_pool.tile([128, 128], FP, tag="h2Tt", bufs=2)
                            nc.vector.tensor_copy(h2T_t[:], ptr[:])
                            nc.vector.tensor_copy(h2T[:, kb, tb * 128:(tb + 1) * 128], h2T_t[:])
                            nc.tensor.matmul(pl[:], h2T_t[:], gw_sb[:, kb, :],
                                             start=(kb == 0), stop=False)
                        nc.tensor.matmul(pl[:], ones1f[:], gb_sb[:], start=False, stop=True)
                        # top-2-of-4 gating -> per-expert combine weights wv[tb]
                        m1 = pt_pool.tile([128, 1], FP, tag="m1", bufs=2)
                        nc.vector.tensor_reduce(m1[:], pl[:], mybir.AxisListType.X, ALU.max)
                        eq1 = pt_pool.tile([128, E], FP, tag="eq1", bufs=2)
                        nc.vector.tensor_scalar(eq1[:], pl[:], m1[:], None, ALU.is_equal)
                        msk = pt_pool.tile([128, E], FP, tag="msk", bufs=2)
                        nc.vector.scalar_tensor_tensor(msk[:], eq1[:], -1e30, pl[:],
                                                       ALU.mult, ALU.add)
                        m2 = pt_pool.tile([128, 1], FP, tag="m2", bufs=2)
                        nc.vector.tensor_reduce(m2[:], msk[:], mybir.AxisListType.X, ALU.max)
                        eq2 = pt_pool.tile([128, E], FP, tag="eq2", bufs=2)
                        nc.vector.tensor_scalar(eq2[:], msk[:], m2[:], None, ALU.is_equal)
                        dd = pt_pool.tile([128, 1], FP, tag="dd", bufs=2)
                        nc.vector.tensor_sub(dd[:], m2[:], m1[:])
                        p2 = pt_pool.tile([128, 1], FP, tag="p2", bufs=2)
                        nc.scalar.activation(p2[:], dd[:], AF.Sigmoid)
                        p1b = pt_pool.tile([128, 1], FP, tag="p1b", bufs=2)
                        nc.scalar.activation(p1b[:], p2[:], AF.Identity, bias=1.0, scale=-1.0)
                        nc.vector.tensor_scalar(wv[tb][:], eq1[:], p1b[:], None, ALU.mult)
                        nc.vector.scalar_tensor_tensor(wv[tb][:], eq2[:], p2[:], wv[tb][:],
                                                       ALU.mult, ALU.add)

            # =======================================================
            # MoE: all 4 experts, own 512 tokens, fully local
            # =======================================================
            with tc.tile_pool(name="moe", bufs=1) as pq:
                # hid as h-pairs for DoubleRow w2: [128, 2, OWN]
                hid = [pq.tile([128, 2, OWN], F8, tag=f"hid{hp}", bufs=1,
                               name=f"hid{hp}") for hp in range(HB // 2)]
                eacc = [pq.tile([128, D], FP, name=f"eacc{t_}", tag=f"eacc{t_}")
                        for t_ in range(OTB)]
                wvs = [pq.tile([128, E], FP, name=f"wvs{t_}", tag=f"wvs{t_}")
                       for t_ in range(OTB)]
                for t_ in range(OTB):
                    nc.vector.tensor_scalar_mul(wvs[t_][:], wv[t_][:], 1.0 / ESCALE)

                HPQ = QW // 128  # h-blocks per eighth = 4
                for e in range(E):
                    with nc.named_scope(f"moe_w1_{e}"), \
                         tc.tile_pool(name=f"psW1_{e}", bufs=1, space="PSUM") as psA:
                        for h in range(HB):
                            g = e * NQ + h // HPQ
                            if h % HPQ == 0:
                                cur = ew1q if g == 0 else nxt
                                if g + 1 < E * NQ:
                                    nxt = load_ew1q(g + 1)
                            ph = psA.tile([128, 512], FP, tag="ph", bufs=3)
                            hc = (h % HPQ) * 128
                            for kb in range(KB):
                                nc.tensor.matmul(ph[:], cur[:, kb, hc:hc + 128],
                                                 h2T[:, kb, :],
                                                 start=(kb == 0), stop=(kb == KB - 1))
                            nc.scalar.activation(hid[h // 2][:, h % 2, :], ph[:], AF.Gelu,
                                                 bias=eb1_sb[:, e * HB + h:e * HB + h + 1])
                    with nc.named_scope(f"moe_w2_{e}"), \
                         tc.tile_pool(name=f"psW2_{e}", bufs=1, space="PSUM") as psB:
                        peo = [[psB.tile([128, 512], FP, tag=f"peo{nb}_{t_}", bufs=1,
                                         name=f"peo{nb}_{t_}") for t_ in range(OTB)]
                               for nb in range(2)]
                        for hp in range(HB // 2):
                            ew2t = pq.tile([128, 2, D], F8, tag="ew2t", bufs=6)
                            nc.gpsimd.dma_start(
                                ew2t[:],
                                dp["ew2"][(e * HB // 2 + hp) * 128:
                                          (e * HB // 2 + hp + 1) * 128, :])
                            for nb in range(2):
                                for t_ in range(OTB):
                                    nc.tensor.matmul(
                                        peo[nb][t_][:],
                                        hid[hp][:, :, t_ * 128:(t_ + 1) * 128],
                                        ew2t[:, :, nb * 512:(nb + 1) * 512],
                                        start=(hp == 0),
                                        stop=(eb2_zero and hp == HB // 2 - 1),
                                        perf_mode=DR)
                        for nb in range(2):
                            for t_ in range(OTB):
                                if not eb2_zero:
                                    nc.tensor.matmul(
                                        peo[nb][t_][:], ones18[:],
                                        eb2_sb[:, e * D + nb * 512: e * D + (nb + 1) * 512],
                                        start=False, stop=True)
                                nc.vector.scalar_tensor_tensor(
                                    eacc[t_][:, nb * 512:(nb + 1) * 512],
                                    peo[nb][t_][:], wvs[t_][:, e:e + 1],
                                    (xmid if e == 0 else eacc)[t_][:, nb * 512:(nb + 1) * 512],
                                    ALU.mult, ALU.add)
                                if e == E - 1:
                                    eng = nc.sync if (nb * OTB + t_) % 2 == 0 else nc.scalar
                                    eng.dma_start(
                                        out_d[t_ * 128:(t_ + 1) * 128,
                                              nb * 512:(nb + 1) * 512],
                                        eacc[t_][:, nb * 512:(nb + 1) * 512])

    nc.compile()
    return nc


def host_prep(inputs):
    """Build the 8 per-core input maps from full inputs."""
    import ml_dtypes
    f32 = np.float32
    bf16 = ml_dtypes.bfloat16
    x = np.ascontiguousarray(np.asarray(inputs["x"], f32).reshape(B * T, D))
    n1 = np.asarray(inputs["norm1_w"], f32)
    n2 = np.asarray(inputs["norm2_w"], f32)

    def pcol(v, nb):  # [nb*128] -> [128, nb], col b = block b
        return np.ascontiguousarray(np.asarray(v, f32).reshape(nb, 128).T)

    ipw = (np.asarray(inputs["in_proj_w"], f32) * n1[:, None]).astype(bf16)
    gwf = np.asarray(inputs["gate_w"], f32) * n2[:, None]          # [D, E]
    gw = np.ascontiguousarray(gwf.reshape(KB, 128, E).swapaxes(0, 1))  # [128,KB,E]
    fp8 = ml_dtypes.float8_e4m3fn
    ew1f = np.asarray(inputs["e_w1"], f32) * n2[None, :, None]     # [E,D,HID]
    ew1 = np.ascontiguousarray(ew1f.reshape(E * D, HID)).astype(bf16)
    # DoubleRow pair layout: row (e,hp,p) holds [two, d]
    ew2f = np.asarray(inputs["e_w2"], f32) * ESCALE                # [E,HID,D]
    ew2 = np.ascontiguousarray(
        ew2f.reshape(E, HB // 2, 2, 128, D).swapaxes(2, 3)
        .reshape(E * HID // 2, 2 * D)).astype(fp8)
    eb1f = np.asarray(inputs["e_b1"], f32)                         # [E, HID]
    # eb1[p, e*HB+h] = e_b1[e, h*128+p]
    eb1 = np.ascontiguousarray(
        eb1f.reshape(E, HB, 128).transpose(2, 0, 1).reshape(128, E * HB))
    eb2 = (np.asarray(inputs["e_b2"], f32).reshape(1, E * D) * ESCALE).astype(fp8)

    def kw(v):  # [INNER, S] -> [128, MB, S]
        return np.ascontiguousarray(
            np.asarray(v, f32).reshape(MB, 128, S).swapaxes(0, 1)).astype(bf16)

    cwf = np.asarray(inputs["conv_w"], f32)[:, 0, :]               # [INNER, 3]
    cw = np.ascontiguousarray(cwf.reshape(MB, 128, 3).swapaxes(0, 1))

    shared = {
        "ipw": ipw,
        "ipb": pcol(inputs["in_proj_b"], 2 * INNER // 128),
        "cw": cw, "cb": pcol(inputs["conv_b"], MB),
        "dtw": kw(inputs["dt_w"]), "dtb": np.asarray(inputs["dt_b"], f32).reshape(S, 1),
        "bpw": kw(inputs["bp_w"]), "bpb": np.asarray(inputs["bp_b"], f32).reshape(S, 1),
        "cpw": kw(inputs["cp_w"]), "cpb": np.asarray(inputs["cp_b"], f32).reshape(S, 1),
        "s2iw": np.asarray(inputs["s2i_w"], f32).astype(bf16),
        "s2ib": pcol(inputs["s2i_b"], MB),
        "Dp": pcol(inputs["D_param"], MB),
        "ow": np.asarray(inputs["out_w"], f32).astype(bf16),
        "ob": np.asarray(inputs["out_b"], f32).reshape(1, D).astype(bf16),
        "gw": gw, "gb": np.asarray(inputs["gate_b"], f32).reshape(1, E),
        "ew1": ew1, "eb1": eb1, "ew2": ew2, "eb2": eb2,
        "ident": np.eye(128, dtype=f32),
        "identb": np.eye(128, dtype=f32).astype(bf16),
        "ones1": np.ones((1, 128), f32).astype(bf16),
        "ones18": np.ones((1, 128), f32).astype(fp8),
        "ones1f": np.ones((1, 128), f32),
    }
    in_maps = []
    for c in range(N_CORES):
        g0 = c * OWN
        if g0 % T == 0:
            x_sh = np.concatenate([np.zeros((HALO, D), f32), x[g0:g0 + OWN]])
        else:
            x_sh = x[g0 - HALO:g0 + OWN]
        m = dict(shared)
        m["x_sh"] = np.ascontiguousarray(x_sh)
        in_maps.append(m)
    return in_maps


def unshard_out(results):
    """results: list of 8 dicts with 'out' [OWN, D]; core c holds global
    tokens [c*512, (c+1)*512)."""
    full = np.concatenate([results[c]["out"] for c in range(N_CORES)], axis=0)
    return full.reshape(B, T, D)


_NC_CACHE = {}


def _get_nc(eb2_zero=True):
    key = ("nc", eb2_zero)
    if key not in _NC_CACHE:
        _NC_CACHE[key] = build(eb2_zero=eb2_zero)
    return _NC_CACHE[key]


def kernel(**inputs) -> np.ndarray:
    """Full-input entry point: shards across 8 NeuronCores, runs the Bass
    kernel SPMD, reassembles the full [2, 2048, 1024] output."""
    import sys, types
    try:  # NTFF profile hook shim (missing antenv.axon_hooks in this image)
        import antenv.axon_hooks  # noqa: F401
    except ImportError:
        try:
            import antenv
            from trn_agent_boot.trn_boot import _ntff_profile_via_ctypes
            mod = types.ModuleType("antenv.axon_hooks")
            try:
                _hook = _ntff_profile_via_ctypes("/opt/axon/libaxon_pjrt.so")
            except Exception:
                _hook = None
            mod.get_axon_ntff_profile_hook = lambda: _hook
            mod.set_axon_ntff_profile_hook = lambda h: None
            sys.modules["antenv.axon_hooks"] = mod
            antenv.axon_hooks = mod
        except Exception:
            pass
    from concourse.bass_utils import run_bass_kernel_spmd

    nc = _get_nc(eb2_zero=not np.any(np.asarray(inputs["e_b2"])))
    in_maps = host_prep(inputs)
    res = run_bass_kernel_spmd(nc, in_maps, core_ids=list(range(N_CORES)))
    out = unshard_out(res.results)
    return out.astype(np.float32)


# revision 69
# speedup vs baseline: 1.0159x; 1.0159x over previous
"""Bass kernel builder for nn_MixtureOfMambaBlock — 8-core SPMD, v2.

Sharding: tokens 8-way, fully local (no collectives). Each core owns 512
contiguous tokens (+128-token halo for conv + scan warmup; the scan decay
(1-dt)~0.5 makes 128 steps numerically exact). The MoE is evaluated densely
for all 4 experts on the core's own 512 tokens (same FLOPs as expert-
parallel, zero communication). Weights replicated, bf16 matmuls, weight
layouts pre-swizzled on host for contiguous DMA.
"""
import numpy as np
import concourse.bass as bass
import concourse.bacc as bacc
import concourse.mybir as mybir
import concourse.tile as tile

FP = mybir.dt.float32
FR = mybir.dt.float32r
BF = mybir.dt.bfloat16
F8 = mybir.dt.float8e4
DR = mybir.MatmulPerfMode.DoubleRow
AF = mybir.ActivationFunctionType
ALU = mybir.AluOpType
ESCALE = 32.0            # fp8 expert-weight scale

B, T, D = 2, 2048, 1024
S, INNER = 64, 2048
E, HID = 4, 4096
OWN, HALO = 512, 128
NH = OWN + HALO          # 640
KB = D // 128            # 8  d-blocks
MB = INNER // 128        # 16 inner-blocks
OTB = OWN // 128         # 4  own-token blocks
HB = HID // 128          # 32 hid-blocks
N_CORES = 8

INPUT_SPECS = {
    "x_sh": ([NH, D], FP),
    "ipw": ([D, 2 * INNER], BF),       # norm1 folded
    "ipb": ([128, 2 * INNER // 128], FP),
    "cw": ([128, MB, 3], FP), "cb": ([128, MB], FP),
    "dtw": ([128, MB, S], BF), "dtb": ([S, 1], FP),
    "bpw": ([128, MB, S], BF), "bpb": ([S, 1], FP),
    "cpw": ([128, MB, S], BF), "cpb": ([S, 1], FP),
    "s2iw": ([S, INNER], BF), "s2ib": ([128, MB], FP),
    "Dp": ([128, MB], FP),
    "ow": ([INNER, D], BF), "ob": ([1, D], BF),
    "gw": ([128, KB, E], FP), "gb": ([1, E], FP),   # norm2 folded into gw
    "ew1": ([E * D, HID], BF),         # norm2 folded; rows e*1024+d
    "eb1": ([128, E * HB], FP),
    "ew2": ([E * HID // 2, 2 * D], F8),  # x32; row (e,hp,p) = [p, two, d]
    "eb2": ([1, E * D], F8),           # x32
    "ident": ([128, 128], FP),
    "identb": ([128, 128], BF),
    "ones1": ([1, 128], BF),
    "ones18": ([1, 128], F8),
    "ones1f": ([1, 128], FP),
}


def build(eb2_zero=True):
    nc = bacc.Bacc("TRN2", target_bir_lowering=False, debug=False,
                   num_devices=N_CORES)
    dp = {}
    for name, (shape, dt) in INPUT_SPECS.items():
        dp[name] = nc.dram_tensor(name, shape, dt, kind="ExternalInput")
    out_d = nc.dram_tensor("out", [OWN, D], FP, kind="ExternalOutput")

    with tile.TileContext(nc) as tc:
        with tc.tile_pool(name="outer", bufs=1) as po:
            # ident first (the rms1 transposes hang on it), then x
            ident = po.tile([128, 128], FP)
            nc.sync.dma_start(ident[:], dp["ident"][:])
            identb = po.tile([128, 128], BF)
            nc.sync.dma_start(identb[:], dp["identb"][:])
            xh = po.tile([128, D], FP, name="xh")
            nc.sync.dma_start(xh[:], dp["x_sh"][0:128, :])
            xo = [po.tile([128, D], FP, name=f"xo{t_}", tag=f"xo{t_}")
                  for t_ in range(OTB)]
            for t_ in range(OTB):
                nc.sync.dma_start(xo[t_][:],
                                  dp["x_sh"][(t_ + 1) * 128:(t_ + 2) * 128, :])
            # pre-warm ACT function tables during the DMA head (earliest-needed last)
            warm = po.tile([1, 1], FP, name="warm")
            for fn in (AF.Gelu, AF.Sigmoid, AF.Sqrt, AF.Square):
                nc.scalar.activation(warm[:], ident[0:1, 0:1], fn)
            # ---------- constants / persistent ----------
            ones1 = po.tile([1, 128], BF)
            nc.sync.dma_start(ones1[:], dp["ones1"][:])
            ones1f = po.tile([1, 128], FP)
            nc.sync.dma_start(ones1f[:], dp["ones1f"][:])
            ob_sb = po.tile([1, D], BF)
            nc.sync.dma_start(ob_sb[:], dp["ob"][:])
            gb_sb = po.tile([1, E], FP)
            nc.sync.dma_start(gb_sb[:], dp["gb"][:])
            if not eb2_zero:
                ones18 = po.tile([1, 128], F8)
                nc.sync.dma_start(ones18[:], dp["ones18"][:])
                eb2_sb = po.tile([1, E * D], F8)
                nc.sync.dma_start(eb2_sb[:], dp["eb2"][:])
            eb1_sb = po.tile([128, E * HB], FP)
            nc.sync.dma_start(eb1_sb[:], dp["eb1"][:])
            gw_sb = po.tile([128, KB, E], FP)
            nc.sync.dma_start(gw_sb[:], dp["gw"][:])

            # persistent activations
            xmid = [po.tile([128, D], FP, name=f"xmid{t_}", tag=f"xmid{t_}")
                    for t_ in range(OTB)]
            h2T = po.tile([128, KB, OWN], BF, name="h2T")
            wv = [po.tile([128, E], FP, name=f"wv{t_}", tag=f"wv{t_}")
                  for t_ in range(OTB)]
            # ew1 streamed in eighths (512 hid cols), double-buffered
            NQ = 8          # eighths per expert
            QW = HID // NQ  # 512

            def load_ew1q(g):  # g = e*NQ + q, into tag g%2
                e, q = divmod(g, NQ)
                t = po.tile([128, KB, QW], BF, tag=f"ew1q{g % 2}",
                            name=f"ew1q{g % 2}")
                for kb in range(KB):
                    nc.sync.dma_start(
                        t[:, kb, :],
                        dp["ew1"][e * D + kb * 128: e * D + (kb + 1) * 128,
                                  q * QW:(q + 1) * QW])
                return t

            # =======================================================
            # MIXER
            # =======================================================
            with (
                tc.tile_pool(name="mixer", bufs=1) as pm,
                tc.tile_pool(name="mixt", bufs=1) as pt_pool,
            ):
                hT = [pm.tile([128, NH], BF, name=f"hT{kb}", tag=f"hT{kb}")
                      for kb in range(KB)]
                xm = [pm.tile([128, NH], BF, name=f"xm{m}", tag=f"xm{m}")
                      for m in range(MB)]

                # ---- rmsnorm1 + transpose to hT (bf16) ----
                with nc.named_scope("rms1"), tc.tile_pool(name="ps1", bufs=1, space="PSUM") as psA:
                    for tb in range(NH // 128):
                        xt = xh if tb == 0 else xo[tb - 1]
                        scr = pt_pool.tile([128, D], FP, tag="scr", bufs=2)
                        sq = pt_pool.tile([128, 1], FP, tag="sq", bufs=2)
                        nc.scalar.activation(scr[:], xt[:], AF.Square, accum_out=sq[:])
                        nr = pt_pool.tile([128, 1], FP, tag="nr", bufs=2)
                        nc.vector.tensor_scalar(nr[:], sq[:], 1.0 / D, 1e-6, ALU.mult, ALU.add)
                        nc.scalar.sqrt(nr[:], nr[:])
                        nc.vector.reciprocal(nr[:], nr[:])
                        h_t = pt_pool.tile([128, D], BF, tag="htb", bufs=2)
                        nc.vector.tensor_scalar(h_t[:], xt[:], nr[:], None, ALU.mult)
                        for kb in range(KB):
                            ptr = psA.tile([128, 128], BF, tag="ptr", bufs=2)
                            nc.tensor.transpose(ptr[:], h_t[:, kb * 128:(kb + 1) * 128],
                                                identb[:])
                            nc.vector.tensor_copy(hT[kb][:, tb * 128:(tb + 1) * 128], ptr[:])

                ipb_sb = pm.tile([128, 2 * INNER // 128], FP)
                nc.sync.dma_start(ipb_sb[:], dp["ipb"][:])
                cb_sb = pm.tile([128, MB], FP)
                nc.sync.dma_start(cb_sb[:], dp["cb"][:])
                cw_sb = pm.tile([128, MB, 3], FP)
                nc.sync.dma_start(cw_sb[:], dp["cw"][:])

                # ---- in_proj (x_main half) + conv + silu ----
                with nc.named_scope("in_proj"), tc.tile_pool(name="ps2", bufs=1, space="PSUM") as psA:
                    for q in range(4):
                        wq = []
                        for kb in range(KB):
                            wt = pt_pool.tile([128, 512], BF, tag=f"wip{kb}", bufs=2,
                                              name=f"wip{kb}")
                            nc.gpsimd.dma_start(
                                wt[:], dp["ipw"][kb * 128:(kb + 1) * 128,
                                                 q * 512:(q + 1) * 512])
                            wq.append(wt)
                        for mi in range(4):
                            m = q * 4 + mi
                            xzp = pt_pool.tile([128, NH + 2], FP, tag="xzp", bufs=2)
                            nc.vector.memset(xzp[:, 0:2], 0.0)
                            for n0, nw in ((0, 512), (512, 128)):
                                px = psA.tile([128, 512], FP, tag="px", bufs=3)
                                for kb in range(KB):
                                    nc.tensor.matmul(px[:, 0:nw],
                                                     wq[kb][:, mi * 128:(mi + 1) * 128],
                                                     hT[kb][:, n0:n0 + nw],
                                                     start=(kb == 0), stop=(kb == KB - 1))
                                nc.scalar.activation(xzp[:, 2 + n0:2 + n0 + nw], px[:, 0:nw],
                                                     AF.Identity, bias=ipb_sb[:, m:m + 1])
                            cv = pt_pool.tile([128, NH], FP, tag="cv", bufs=2)
                            nc.vector.tensor_scalar(cv[:], xzp[:, 0:NH], cw_sb[:, m, 0:1],
                                                    None, ALU.mult)
                            nc.vector.scalar_tensor_tensor(cv[:], xzp[:, 1:1 + NH],
                                                           cw_sb[:, m, 1:2], cv[:],
                                                           ALU.mult, ALU.add)
                            nc.vector.scalar_tensor_tensor(cv[:], xzp[:, 2:2 + NH],
                                                           cw_sb[:, m, 2:3], cv[:],
                                                           ALU.mult, ALU.add)
                            sgc = pt_pool.tile([128, NH], FP, tag="sgc", bufs=2)
                            nc.scalar.activation(sgc[:], cv[:], AF.Sigmoid, bias=cb_sb[:, m:m + 1])
                            nc.vector.scalar_tensor_tensor(xm[m][:], cv[:], cb_sb[:, m:m + 1],
                                                           sgc[:], ALU.add, ALU.mult)

                dtb_sb = pm.tile([S, 1], FP)
                nc.sync.dma_start(dtb_sb[:], dp["dtb"][:])
                bpb_sb = pm.tile([S, 1], FP)
                nc.sync.dma_start(bpb_sb[:], dp["bpb"][:])
                cpb_sb = pm.tile([S, 1], FP)
                nc.sync.dma_start(cpb_sb[:], dp["cpb"][:])
                dtw_sb = pm.tile([128, MB, S], BF)
                nc.sync.dma_start(dtw_sb[:], dp["dtw"][:])
                bpw_sb = pm.tile([128, MB, S], BF)
                nc.sync.dma_start(bpw_sb[:], dp["bpw"][:])
                cpw_sb = pm.tile([128, MB, S], BF)
                nc.sync.dma_start(cpw_sb[:], dp["cpw"][:])
                ew1q = load_ew1q(0)

                # dedicated PSUM pool for premix gate matmuls, opened BEFORE the
                # scan pool so its banks never alias the scan/LN psums — lets the
                # pg prefill run during the serial scan DVE chain.
                from contextlib import ExitStack
                _pgstack = ExitStack()
                psPG = _pgstack.enter_context(
                    tc.tile_pool(name="psPG", bufs=1, space="PSUM"))
                wqs = {}
                pgs = {}

                def emit_pg(m):
                    q, mi = divmod(m, 4)
                    if mi == 0:
                        wq = []
                        for kb in range(KB):
                            wt = pt_pool.tile([128, 512], BF, tag=f"wip{kb}", bufs=2,
                                              name=f"wipg{kb}")
                            nc.gpsimd.dma_start(
                                wt[:], dp["ipw"][kb * 128:(kb + 1) * 128,
                                                 2048 + q * 512:2048 + (q + 1) * 512])
                            wq.append(wt)
                        wqs[q] = wq
                    pg = psPG.tile([128, 512], FP, tag="pg", bufs=4)
                    for kb in range(KB):
                        nc.tensor.matmul(pg[:], wqs[q][kb][:, mi * 128:(mi + 1) * 128],
                                         hT[kb][:, HALO:NH],
                                         start=(kb == 0), stop=(kb == KB - 1))
                    pgs[m] = pg

                # ---- dt/B/C projections + scan ----
                with nc.named_scope("scan"), tc.tile_pool(name="ps3", bufs=1, space="PSUM") as psA:
                    dt_t = pt_pool.tile([S, NH], FP, tag="dt")
                    a_t = pt_pool.tile([S, NH], FP, tag="a")
                    b_t = pt_pool.tile([S, NH], FP, tag="b")
                    c_t = pt_pool.tile([S, NH], FP, tag="c")
                    for n0, nw in ((0, 512), (512, 128)):
                        for wsb, bias_sb, dst, fn in (
                            (dtw_sb, dtb_sb, dt_t, AF.Sigmoid),
                            (cpw_sb, cpb_sb, c_t, AF.Identity),
                        ):
                            pz = psA.tile([S, 512], FP, tag="pz", bufs=2)
                            for kb in range(MB):
                                nc.tensor.matmul(pz[:, 0:nw], wsb[:, kb, :],
                                                 xm[kb][:, n0:n0 + nw],
                                                 start=(kb == 0), stop=(kb == MB - 1))
                            nc.scalar.activation(dst[:, n0:n0 + nw], pz[:, 0:nw], fn,
                                                 bias=bias_sb[:])
                        # b needs dt -> separate pass
                        pz = psA.tile([S, 512], FP, tag="pz", bufs=2)
                        for kb in range(MB):
                            nc.tensor.matmul(pz[:, 0:nw], bpw_sb[:, kb, :],
                                             xm[kb][:, n0:n0 + nw],
                                             start=(kb == 0), stop=(kb == MB - 1))
                        nc.vector.scalar_tensor_tensor(b_t[:, n0:n0 + nw], pz[:, 0:nw],
                                                       bpb_sb[:], dt_t[:, n0:n0 + nw],
                                                       ALU.add, ALU.mult)
                    nc.scalar.activation(a_t[:], dt_t[:], AF.Identity, bias=1.0, scale=-1.0)
                    st_t = pt_pool.tile([S, NH], FP, tag="st")
                    nc.vector.tensor_tensor_scan(st_t[:], a_t[:], b_t[:], 0.0,
                                                 ALU.mult, ALU.add)
                    y_t = pt_pool.tile([S, OWN], FP, tag="dt", name="y_t")
                    nc.vector.tensor_mul(y_t[:], c_t[:, HALO:NH], st_t[:, HALO:NH])

                # prefill gate matmuls: fills the PE while the scan DVE chain runs
                emit_pg(0)
                emit_pg(1)
                emit_pg(2)

                # ---- layernorm over S (transpose - LN - transpose back) ----
                with nc.named_scope("ln"), tc.tile_pool(name="ps4", bufs=1, space="PSUM") as psA:
                    yln = pt_pool.tile([S, OWN], BF, tag="a", name="yln")
                    for i in range(OTB):
                        ptr = psA.tile([128, 128], FP, tag="ptr", bufs=2)
                        nc.tensor.transpose(ptr[:, 0:S], y_t[:, i * 128:(i + 1) * 128],
                                            ident[0:S, 0:S])
                        yT = pt_pool.tile([128, S], FP, tag="yT", bufs=2)
                        nc.vector.tensor_copy(yT[:], ptr[:, 0:S])
                        mu = pt_pool.tile([128, 1], FP, tag="mu", bufs=2)
                        nc.vector.tensor_reduce(mu[:], yT[:], mybir.AxisListType.X, ALU.add)
                        nc.vector.tensor_scalar_mul(mu[:], mu[:], 1.0 / S)
                        xc = pt_pool.tile([128, S], FP, tag="xc", bufs=2)
                        nc.vector.tensor_scalar_sub(xc[:], yT[:], mu[:])
                        scr2 = pt_pool.tile([128, S], FP, tag="scr2", bufs=2)
                        vv = pt_pool.tile([128, 1], FP, tag="vv", bufs=2)
                        nc.scalar.activation(scr2[:], xc[:], AF.Square, accum_out=vv[:])
                        nc.vector.tensor_scalar(vv[:], vv[:], 1.0 / S, 1e-5, ALU.mult, ALU.add)
                        nc.scalar.sqrt(vv[:], vv[:])
                        nc.vector.reciprocal(vv[:], vv[:])
                        nc.vector.tensor_scalar_mul(xc[:], xc[:], vv[:])
                        ptr2 = psA.tile([128, 128], FP, tag="ptr2", bufs=2)
                        nc.tensor.transpose(ptr2[0:S, :], xc[:], ident[:])
                        nc.vector.tensor_copy(yln[:, i * 128:(i + 1) * 128], ptr2[0:S, :])

                s2ib_sb = pm.tile([128, MB], FP)
                nc.sync.dma_start(s2ib_sb[:], dp["s2ib"][:])
                Dp_sb = pm.tile([128, MB], FP)
                nc.sync.dma_start(Dp_sb[:], dp["Dp"][:])
                s2iw_sb = pm.tile([S, INNER], BF)
                nc.sync.dma_start(s2iw_sb[:], dp["s2iw"][:])

                # ---- s2i + gate sigmoid + pre_out assembly ----
                with nc.named_scope("premix"), tc.tile_pool(name="ps5", bufs=1, space="PSUM") as psA:
                    pre = []
                    for m in range(MB):
                        if m + 3 < MB:
                            emit_pg(m + 3)
                        ps = psA.tile([128, 512], FP, tag="ps", bufs=2)
                        nc.tensor.matmul(ps[:], s2iw_sb[:, m * 128:(m + 1) * 128], yln[:],
                                         start=True, stop=True)
                        sg = pt_pool.tile([128, OWN], FP, tag="sg", bufs=2)
                        nc.scalar.activation(sg[:], pgs.pop(m)[:], AF.Sigmoid,
                                             bias=ipb_sb[:, MB + m:MB + m + 1])
                        tmp = pt_pool.tile([128, OWN], FP, tag="tmp", bufs=2)
                        nc.vector.tensor_scalar(tmp[:], xm[m][:, HALO:NH],
                                                Dp_sb[:, m:m + 1], None, ALU.mult)
                        nc.vector.scalar_tensor_tensor(tmp[:], ps[:], s2ib_sb[:, m:m + 1],
                                                       tmp[:], ALU.add, ALU.add)
                        pre_m = pm.tile([128, OWN], BF, tag=f"xm{m}", name=f"pre{m}")
                        nc.vector.tensor_mul(pre_m[:], tmp[:], sg[:])
                        pre.append(pre_m)
                _pgstack.close()

                # ---- out projection + residual + rms2 + h2T + gating ----
                with nc.named_scope("outproj"), tc.tile_pool(name="ps6", bufs=1, space="PSUM") as psA:
                    for nb in range(2):
                        po_t = [psA.tile([128, 512], FP, tag=f"po{t_}", bufs=1,
                                         name=f"po{nb}_{t_}") for t_ in range(OTB)]
                        for kb in range(MB):
                            owt = pt_pool.tile([128, 512], BF, tag="owt", bufs=4)
                            nc.gpsimd.dma_start(
                                owt[:], dp["ow"][kb * 128:(kb + 1) * 128,
                                                 nb * 512:(nb + 1) * 512])
                            for t_ in range(OTB):
                                nc.tensor.matmul(po_t[t_][:],
                                                 pre[kb][:, t_ * 128:(t_ + 1) * 128],
                                                 owt[:], start=(kb == 0), stop=False)
                        for t_ in range(OTB):
                            nc.tensor.matmul(po_t[t_][:], ones1[:],
                                             ob_sb[:, nb * 512:(nb + 1) * 512],
                                             start=False, stop=True)
                            nc.vector.tensor_add(xmid[t_][:, nb * 512:(nb + 1) * 512],
                                                 po_t[t_][:],
                                                 xo[t_][:, nb * 512:(nb + 1) * 512])
                    for tb in range(OTB):
                        # rms2 for this tb
                        scr = pt_pool.tile([128, D], FP, tag="scr", bufs=2)
                        sq = pt_pool.tile([128, 1], FP, tag="sq", bufs=2)
                        nc.scalar.activation(scr[:], xmid[tb][:], AF.Square, accum_out=sq[:])
                        nr = pt_pool.tile([128, 1], FP, tag="nr", bufs=2)
                        nc.vector.tensor_scalar(nr[:], sq[:], 1.0 / D, 1e-6, ALU.mult, ALU.add)
                        nc.scalar.sqrt(nr[:], nr[:])
                        nc.vector.reciprocal(nr[:], nr[:])
                        h2 = pt_pool.tile([128, D], FP, tag="xt", bufs=2, name="h2")
                        nc.vector.tensor_scalar(h2[:], xmid[tb][:], nr[:], None, ALU.mult)
                        pl = psA.tile([128, E], FP, tag="pl", bufs=2)
                        for kb in range(KB):
                            ptr = psA.tile([128, 128], FP, tag="ptr", bufs=2)
                            nc.tensor.transpose(ptr[:], h2[:, kb * 128:(kb + 1) * 128], ident[:])
                            h2T_t = pt_pool.tile([128, 128], FP, tag="h2Tt", bufs=2)
                            nc.vector.tensor_copy(h2T_t[:], ptr[:])
                            nc.vector.tensor_copy(h2T[:, kb, tb * 128:(tb + 1) * 128], h2T_t[:])
                            nc.tensor.matmul(pl[:], h2T_t[:], gw_sb[:, kb, :],
                                             start=(kb == 0), stop=False)
                        nc.tensor.matmul(pl[:], ones1f[:], gb_sb[:], start=False, stop=True)
                        # top-2-of-4 gating -> per-expert combine weights wv[tb]
                        m1 = pt_pool.tile([128, 1], FP, tag="m1", bufs=2)
                        nc.vector.tensor_reduce(m1[:], pl[:], mybir.AxisListType.X, ALU.max)
                        eq1 = pt_pool.tile([128, E], FP, tag="eq1", bufs=2)
                        nc.vector.tensor_scalar(eq1[:], pl[:], m1[:], None, ALU.is_equal)
                        msk = pt_pool.tile([128, E], FP, tag="msk", bufs=2)
                        nc.vector.scalar_tensor_tensor(msk[:], eq1[:], -1e30, pl[:],
                                                       ALU.mult, ALU.add)
                        m2 = pt_pool.tile([128, 1], FP, tag="m2", bufs=2)
                        nc.vector.tensor_reduce(m2[:], msk[:], mybir.AxisListType.X, ALU.max)
                        eq2 = pt_pool.tile([128, E], FP, tag="eq2", bufs=2)
                        nc.vector.tensor_scalar(eq2[:], msk[:], m2[:], None, ALU.is_equal)
                        dd = pt_pool.tile([128, 1], FP, tag="dd", bufs=2)
                        nc.vector.tensor_sub(dd[:], m2[:], m1[:])
                        p2 = pt_pool.tile([128, 1], FP, tag="p2", bufs=2)
                        nc.scalar.activation(p2[:], dd[:], AF.Sigmoid)
                        p1b = pt_pool.tile([128, 1], FP, tag="p1b", bufs=2)
                        nc.scalar.activation(p1b[:], p2[:], AF.Identity, bias=1.0, scale=-1.0)
                        nc.vector.tensor_scalar(wv[tb][:], eq1[:], p1b[:], None, ALU.mult)
                        nc.vector.scalar_tensor_tensor(wv[tb][:], eq2[:], p2[:], wv[tb][:],
                                                       ALU.mult, ALU.add)

            # =======================================================
            # MoE: all 4 experts, own 512 tokens, fully local
            # =======================================================
            with tc.tile_pool(name="moe", bufs=1) as pq:
                # hid as h-pairs for DoubleRow w2: [128, 2, OWN]
                hid = [pq.tile([128, 2, OWN], F8, tag=f"hid{hp}", bufs=1,
                               name=f"hid{hp}") for hp in range(HB // 2)]
                eacc = [pq.tile([128, D], FP, name=f"eacc{t_}", tag=f"eacc{t_}")
                        for t_ in range(OTB)]
                wvs = [pq.tile([128, E], FP, name=f"wvs{t_}", tag=f"wvs{t_}")
                       for t_ in range(OTB)]
                for t_ in range(OTB):
                    nc.vector.tensor_scalar_mul(wvs[t_][:], wv[t_][:], 1.0 / ESCALE)

                HPQ = QW // 128  # h-blocks per eighth = 4
                for e in range(E):
                    with nc.named_scope(f"moe_w1_{e}"), \
                         tc.tile_pool(name=f"psW1_{e}", bufs=1, space="PSUM") as psA:
                        for h in range(HB):
                            g = e * NQ + h // HPQ
                            if h % HPQ == 0:
                                cur = ew1q if g == 0 else nxt
                                if g + 1 < E * NQ:
                                    nxt = load_ew1q(g + 1)
                            ph = psA.tile([128, 512], FP, tag="ph", bufs=3)
                            hc = (h % HPQ) * 128
                            for kb in range(KB):
                                nc.tensor.matmul(ph[:], cur[:, kb, hc:hc + 128],
                                                 h2T[:, kb, :],
                                                 start=(kb == 0), stop=(kb == KB - 1))
                            nc.scalar.activation(hid[h // 2][:, h % 2, :], ph[:], AF.Gelu,
                                                 bias=eb1_sb[:, e * HB + h:e * HB + h + 1])
                    with nc.named_scope(f"moe_w2_{e}"), \
                         tc.tile_pool(name=f"psW2_{e}", bufs=1, space="PSUM") as psB:
                        peo = [[psB.tile([128, 512], FP, tag=f"peo{nb}_{t_}", bufs=1,
                                         name=f"peo{nb}_{t_}") for t_ in range(OTB)]
                               for nb in range(2)]
                        for hp in range(HB // 2):
                            ew2t = pq.tile([128, 2, D], F8, tag="ew2t", bufs=6)
                            nc.gpsimd.dma_start(
                                ew2t[:],
                                dp["ew2"][(e * HB // 2 + hp) * 128:
                                          (e * HB // 2 + hp + 1) * 128, :])
                            for nb in range(2):
                                for t_ in range(OTB):
                                    nc.tensor.matmul(
                                        peo[nb][t_][:],
                                        hid[hp][:, :, t_ * 128:(t_ + 1) * 128],
                                        ew2t[:, :, nb * 512:(nb + 1) * 512],
                                        start=(hp == 0),
                                        stop=(eb2_zero and hp == HB // 2 - 1),
                                        perf_mode=DR)
                        for nb in range(2):
                            for t_ in range(OTB):
                                if not eb2_zero:
                                    nc.tensor.matmul(
                                        peo[nb][t_][:], ones18[:],
                                        eb2_sb[:, e * D + nb * 512: e * D + (nb + 1) * 512],
                                        start=False, stop=True)
                                nc.vector.scalar_tensor_tensor(
                                    eacc[t_][:, nb * 512:(nb + 1) * 512],
                                    peo[nb][t_][:], wvs[t_][:, e:e + 1],
                                    (xmid if e == 0 else eacc)[t_][:, nb * 512:(nb + 1) * 512],
                                    ALU.mult, ALU.add)
                                if e == E - 1:
                                    eng = nc.sync if (nb * OTB + t_) % 2 == 0 else nc.scalar
                                    eng.dma_start(
                                        out_d[t_ * 128:(t_ + 1) * 128,
                                              nb * 512:(nb + 1) * 512],
                                        eacc[t_][:, nb * 512:(nb + 1) * 512])

    nc.compile()
    return nc


def host_prep(inputs):
    """Build the 8 per-core input maps from full inputs."""
    import ml_dtypes
    f32 = np.float32
    bf16 = ml_dtypes.bfloat16
    x = np.ascontiguousarray(np.asarray(inputs["x"], f32).reshape(B * T, D))
    n1 = np.asarray(inputs["norm1_w"], f32)
    n2 = np.asarray(inputs["norm2_w"], f32)

    def pcol(v, nb):  # [nb*128] -> [128, nb], col b = block b
        return np.ascontiguousarray(np.asarray(v, f32).reshape(nb, 128).T)

    ipw = (np.asarray(inputs["in_proj_w"], f32) * n1[:, None]).astype(bf16)
    gwf = np.asarray(inputs["gate_w"], f32) * n2[:, None]          # [D, E]
    gw = np.ascontiguousarray(gwf.reshape(KB, 128, E).swapaxes(0, 1))  # [128,KB,E]
    fp8 = ml_dtypes.float8_e4m3fn
    ew1f = np.asarray(inputs["e_w1"], f32) * n2[None, :, None]     # [E,D,HID]
    ew1 = np.ascontiguousarray(ew1f.reshape(E * D, HID)).astype(bf16)
    # DoubleRow pair layout: row (e,hp,p) holds [two, d]
    ew2f = np.asarray(inputs["e_w2"], f32) * ESCALE                # [E,HID,D]
    ew2 = np.ascontiguousarray(
        ew2f.reshape(E, HB // 2, 2, 128, D).swapaxes(2, 3)
        .reshape(E * HID // 2, 2 * D)).astype(fp8)
    eb1f = np.asarray(inputs["e_b1"], f32)                         # [E, HID]
    # eb1[p, e*HB+h] = e_b1[e, h*128+p]
    eb1 = np.ascontiguousarray(
        eb1f.reshape(E, HB, 128).transpose(2, 0, 1).reshape(128, E * HB))
    eb2 = (np.asarray(inputs["e_b2"], f32).reshape(1, E * D) * ESCALE).astype(fp8)

    def kw(v):  # [INNER, S] -> [128, MB, S]
        return np.ascontiguousarray(
            np.asarray(v, f32).reshape(MB, 128, S).swapaxes(0, 1)).astype(bf16)

    cwf = np.asarray(inputs["conv_w"], f32)[:, 0, :]               # [INNER, 3]
    cw = np.ascontiguousarray(cwf.reshape(MB, 128, 3).swapaxes(0, 1))

    shared = {
        "ipw": ipw,
        "ipb": pcol(inputs["in_proj_b"], 2 * INNER // 128),
        "cw": cw, "cb": pcol(inputs["conv_b"], MB),
        "dtw": kw(inputs["dt_w"]), "dtb": np.asarray(inputs["dt_b"], f32).reshape(S, 1),
        "bpw": kw(inputs["bp_w"]), "bpb": np.asarray(inputs["bp_b"], f32).reshape(S, 1),
        "cpw": kw(inputs["cp_w"]), "cpb": np.asarray(inputs["cp_b"], f32).reshape(S, 1),
        "s2iw": np.asarray(inputs["s2i_w"], f32).astype(bf16),
        "s2ib": pcol(inputs["s2i_b"], MB),
        "Dp": pcol(inputs["D_param"], MB),
        "ow": np.asarray(inputs["out_w"], f32).astype(bf16),
        "ob": np.asarray(inputs["out_b"], f32).reshape(1, D).astype(bf16),
        "gw": gw, "gb": np.asarray(inputs["gate_b"], f32).reshape(1, E),
        "ew1": ew1, "eb1": eb1, "ew2": ew2, "eb2": eb2,
        "ident": np.eye(128, dtype=f32),
        "identb": np.eye(128, dtype=f32).astype(bf16),
        "ones1": np.ones((1, 128), f32).astype(bf16),
        "ones18": np.ones((1, 128), f32).astype(fp8),
        "ones1f": np.ones((1, 128), f32),
    }
    in_maps = []
    for c in range(N_CORES):
        g0 = c * OWN
        if g0 % T == 0:
            x_sh = np.concatenate([np.zeros((HALO, D), f32), x[g0:g0 + OWN]])
        else:
            x_sh = x[g0 - HALO:g0 + OWN]
        m = dict(shared)
        m["x_sh"] = np.ascontiguousarray(x_sh)
        in_maps.append(m)
    return in_maps


def unshard_out(results):
    """results: list of 8 dicts with 'out' [OWN, D]; core c holds global
    tokens [c*512, (c+1)*512)."""
    full = np.concatenate([results[c]["out"] for c in range(N_CORES)], axis=0)
    return full.reshape(B, T, D)


_NC_CACHE = {}


def _get_nc(eb2_zero=True):
    key = ("nc", eb2_zero)
    if key not in _NC_CACHE:
        _NC_CACHE[key] = build(eb2_zero=eb2_zero)
    return _NC_CACHE[key]


def kernel(**inputs) -> np.ndarray:
    """Full-input entry point: shards across 8 NeuronCores, runs the Bass
    kernel SPMD, reassembles the full [2, 2048, 1024] output."""
    import sys, types
    try:  # NTFF profile hook shim (missing antenv.axon_hooks in this image)
        import antenv.axon_hooks  # noqa: F401
    except ImportError:
        try:
            import antenv
            from trn_agent_boot.trn_boot import _ntff_profile_via_ctypes
            mod = types.ModuleType("antenv.axon_hooks")
            try:
                _hook = _ntff_profile_via_ctypes("/opt/axon/libaxon_pjrt.so")
            except Exception:
                _hook = None
            mod.get_axon_ntff_profile_hook = lambda: _hook
            mod.set_axon_ntff_profile_hook = lambda h: None
            sys.modules["antenv.axon_hooks"] = mod
            antenv.axon_hooks = mod
        except Exception:
            pass
    from concourse.bass_utils import run_bass_kernel_spmd

    nc = _get_nc(eb2_zero=not np.any(np.asarray(inputs["e_b2"])))
    in_maps = host_prep(inputs)
    res = run_bass_kernel_spmd(nc, in_maps, core_ids=list(range(N_CORES)))
    out = unshard_out(res.results)
    return out.astype(np.float32)


# revision 70
# speedup vs baseline: 1.0383x; 1.0221x over previous
"""Bass kernel builder for nn_MixtureOfMambaBlock — 8-core SPMD, v2.

Sharding: tokens 8-way, fully local (no collectives). Each core owns 512
contiguous tokens (+128-token halo for conv + scan warmup; the scan decay
(1-dt)~0.5 makes 128 steps numerically exact). The MoE is evaluated densely
for all 4 experts on the core's own 512 tokens (same FLOPs as expert-
parallel, zero communication). Weights replicated, bf16 matmuls, weight
layouts pre-swizzled on host for contiguous DMA.
"""
import numpy as np
import concourse.bass as bass
import concourse.bacc as bacc
import concourse.mybir as mybir
import concourse.tile as tile

FP = mybir.dt.float32
FR = mybir.dt.float32r
BF = mybir.dt.bfloat16
F8 = mybir.dt.float8e4
DR = mybir.MatmulPerfMode.DoubleRow
AF = mybir.ActivationFunctionType
ALU = mybir.AluOpType
ESCALE = 32.0            # fp8 expert-weight scale

B, T, D = 2, 2048, 1024
S, INNER = 64, 2048
E, HID = 4, 4096
OWN, HALO = 512, 128
NH = OWN + HALO          # 640
KB = D // 128            # 8  d-blocks
MB = INNER // 128        # 16 inner-blocks
OTB = OWN // 128         # 4  own-token blocks
HB = HID // 128          # 32 hid-blocks
N_CORES = 8

INPUT_SPECS = {
    "x_sh": ([NH, D], FP),
    "ipw": ([D, 2 * INNER], BF),       # norm1 folded
    "ipb": ([128, 2 * INNER // 128], FP),
    "cw": ([128, MB, 3], FP), "cb": ([128, MB], FP),
    "dtw": ([128, MB, S], BF), "dtb": ([S, 1], FP),
    "bpw": ([128, MB, S], BF), "bpb": ([S, 1], FP),
    "cpw": ([128, MB, S], BF), "cpb": ([S, 1], FP),
    "s2iw": ([S, INNER], BF), "s2ib": ([128, MB], FP),
    "Dp": ([128, MB], FP),
    "ow": ([INNER, D], BF), "ob": ([1, D], BF),
    "gw": ([128, KB, E], FP), "gb": ([1, E], FP),   # norm2 folded into gw
    "ew1": ([E * D, HID], BF),         # norm2 folded; rows e*1024+d
    "eb1": ([128, E * HB], FP),
    "ew2": ([E * HID // 2, 2 * D], F8),  # x32; row (e,hp,p) = [p, two, d]
    "eb2": ([1, E * D], F8),           # x32
    "ident": ([128, 128], FP),
    "identb": ([128, 128], BF),
    "ones1": ([1, 128], BF),
    "ones18": ([1, 128], F8),
    "ones1f": ([1, 128], FP),
}


def build(eb2_zero=True):
    nc = bacc.Bacc("TRN2", target_bir_lowering=False, debug=False,
                   num_devices=N_CORES)
    dp = {}
    for name, (shape, dt) in INPUT_SPECS.items():
        dp[name] = nc.dram_tensor(name, shape, dt, kind="ExternalInput")
    out_d = nc.dram_tensor("out", [OWN, D], FP, kind="ExternalOutput")

    with tile.TileContext(nc) as tc:
        with tc.tile_pool(name="outer", bufs=1) as po:
            # ident first (the rms1 transposes hang on it), then x
            ident = po.tile([128, 128], FP)
            nc.sync.dma_start(ident[:], dp["ident"][:])
            identb = po.tile([128, 128], BF)
            nc.sync.dma_start(identb[:], dp["identb"][:])
            xh = po.tile([128, D], FP, name="xh")
            nc.sync.dma_start(xh[:], dp["x_sh"][0:128, :])
            xo = [po.tile([128, D], FP, name=f"xo{t_}", tag=f"xo{t_}")
                  for t_ in range(OTB)]
            for t_ in range(OTB):
                nc.sync.dma_start(xo[t_][:],
                                  dp["x_sh"][(t_ + 1) * 128:(t_ + 2) * 128, :])
            # pre-warm ACT function tables during the DMA head (earliest-needed last)
            warm = po.tile([1, 1], FP, name="warm")
            for fn in (AF.Gelu, AF.Sigmoid, AF.Sqrt, AF.Square):
                nc.scalar.activation(warm[:], ident[0:1, 0:1], fn)
            # ---------- constants / persistent ----------
            ones1 = po.tile([1, 128], BF)
            nc.sync.dma_start(ones1[:], dp["ones1"][:])
            ones1f = po.tile([1, 128], FP)
            nc.sync.dma_start(ones1f[:], dp["ones1f"][:])
            ob_sb = po.tile([1, D], BF)
            nc.sync.dma_start(ob_sb[:], dp["ob"][:])
            gb_sb = po.tile([1, E], FP)
            nc.sync.dma_start(gb_sb[:], dp["gb"][:])
            if not eb2_zero:
                ones18 = po.tile([1, 128], F8)
                nc.sync.dma_start(ones18[:], dp["ones18"][:])
                eb2_sb = po.tile([1, E * D], F8)
                nc.sync.dma_start(eb2_sb[:], dp["eb2"][:])
            eb1_sb = po.tile([128, E * HB], FP)
            nc.sync.dma_start(eb1_sb[:], dp["eb1"][:])
            gw_sb = po.tile([128, KB, E], FP)
            nc.sync.dma_start(gw_sb[:], dp["gw"][:])

            # persistent activations
            xmid = [po.tile([128, D], FP, name=f"xmid{t_}", tag=f"xmid{t_}")
                    for t_ in range(OTB)]
            h2T = po.tile([128, KB, OWN], BF, name="h2T")
            wv = [po.tile([128, E], FP, name=f"wv{t_}", tag=f"wv{t_}")
                  for t_ in range(OTB)]
            # ew1 streamed in eighths (512 hid cols), double-buffered
            NQ = 8          # eighths per expert
            QW = HID // NQ  # 512

            def load_ew1q(g):  # g = e*NQ + q, into tag g%2
                e, q = divmod(g, NQ)
                t = po.tile([128, KB, QW], BF, tag=f"ew1q{g % 2}",
                            name=f"ew1q{g % 2}")
                for kb in range(KB):
                    nc.sync.dma_start(
                        t[:, kb, :],
                        dp["ew1"][e * D + kb * 128: e * D + (kb + 1) * 128,
                                  q * QW:(q + 1) * QW])
                return t

            # =======================================================
            # MIXER
            # =======================================================
            with (
                tc.tile_pool(name="mixer", bufs=1) as pm,
                tc.tile_pool(name="mixt", bufs=1) as pt_pool,
            ):
                hT = [pm.tile([128, NH], BF, name=f"hT{kb}", tag=f"hT{kb}")
                      for kb in range(KB)]
                xm = [pm.tile([128, NH], BF, name=f"xm{m}", tag=f"xm{m}")
                      for m in range(MB)]

                # ---- rmsnorm1 + transpose to hT (bf16) ----
                with nc.named_scope("rms1"), tc.tile_pool(name="ps1", bufs=1, space="PSUM") as psA:
                    for tb in range(NH // 128):
                        xt = xh if tb == 0 else xo[tb - 1]
                        scr = pt_pool.tile([128, D], FP, tag="scr", bufs=2)
                        sq = pt_pool.tile([128, 1], FP, tag="sq", bufs=2)
                        nc.scalar.activation(scr[:], xt[:], AF.Square, accum_out=sq[:])
                        nr = pt_pool.tile([128, 1], FP, tag="nr", bufs=2)
                        nc.vector.tensor_scalar(nr[:], sq[:], 1.0 / D, 1e-6, ALU.mult, ALU.add)
                        nc.scalar.sqrt(nr[:], nr[:])
                        nc.vector.reciprocal(nr[:], nr[:])
                        h_t = pt_pool.tile([128, D], BF, tag="htb", bufs=2)
                        nc.vector.tensor_scalar(h_t[:], xt[:], nr[:], None, ALU.mult)
                        for kb in range(KB):
                            ptr = psA.tile([128, 128], BF, tag="ptr", bufs=2)
                            nc.tensor.transpose(ptr[:], h_t[:, kb * 128:(kb + 1) * 128],
                                                identb[:])
                            nc.vector.tensor_copy(hT[kb][:, tb * 128:(tb + 1) * 128], ptr[:])

                ipb_sb = pm.tile([128, 2 * INNER // 128], FP)
                nc.sync.dma_start(ipb_sb[:], dp["ipb"][:])
                cb_sb = pm.tile([128, MB], FP)
                nc.sync.dma_start(cb_sb[:], dp["cb"][:])
                cw_sb = pm.tile([128, MB, 3], FP)
                nc.sync.dma_start(cw_sb[:], dp["cw"][:])

                # ---- in_proj (x_main half) + conv + silu ----
                with nc.named_scope("in_proj"), tc.tile_pool(name="ps2", bufs=1, space="PSUM") as psA:
                    for q in range(4):
                        wq = []
                        for kb in range(KB):
                            wt = pt_pool.tile([128, 512], BF, tag=f"wip{kb}", bufs=2,
                                              name=f"wip{kb}")
                            nc.gpsimd.dma_start(
                                wt[:], dp["ipw"][kb * 128:(kb + 1) * 128,
                                                 q * 512:(q + 1) * 512])
                            wq.append(wt)
                        for mi in range(4):
                            m = q * 4 + mi
                            xzp = pt_pool.tile([128, NH + 2], FP, tag="xzp", bufs=2)
                            nc.vector.memset(xzp[:, 0:2], 0.0)
                            for n0, nw in ((0, 512), (512, 128)):
                                px = psA.tile([128, 512], FP, tag="px", bufs=3)
                                for kb in range(KB):
                                    nc.tensor.matmul(px[:, 0:nw],
                                                     wq[kb][:, mi * 128:(mi + 1) * 128],
                                                     hT[kb][:, n0:n0 + nw],
                                                     start=(kb == 0), stop=(kb == KB - 1))
                                nc.scalar.activation(xzp[:, 2 + n0:2 + n0 + nw], px[:, 0:nw],
                                                     AF.Identity, bias=ipb_sb[:, m:m + 1])
                            cv = pt_pool.tile([128, NH], FP, tag="cv", bufs=2)
                            nc.vector.tensor_scalar(cv[:], xzp[:, 0:NH], cw_sb[:, m, 0:1],
                                                    None, ALU.mult)
                            nc.vector.scalar_tensor_tensor(cv[:], xzp[:, 1:1 + NH],
                                                           cw_sb[:, m, 1:2], cv[:],
                                                           ALU.mult, ALU.add)
                            nc.vector.scalar_tensor_tensor(cv[:], xzp[:, 2:2 + NH],
                                                           cw_sb[:, m, 2:3], cv[:],
                                                           ALU.mult, ALU.add)
                            sgc = pt_pool.tile([128, NH], FP, tag="sgc", bufs=2)
                            nc.scalar.activation(sgc[:], cv[:], AF.Sigmoid, bias=cb_sb[:, m:m + 1])
                            nc.vector.scalar_tensor_tensor(xm[m][:], cv[:], cb_sb[:, m:m + 1],
                                                           sgc[:], ALU.add, ALU.mult)

                dtb_sb = pm.tile([S, 1], FP)
                nc.sync.dma_start(dtb_sb[:], dp["dtb"][:])
                bpb_sb = pm.tile([S, 1], FP)
                nc.sync.dma_start(bpb_sb[:], dp["bpb"][:])
                cpb_sb = pm.tile([S, 1], FP)
                nc.sync.dma_start(cpb_sb[:], dp["cpb"][:])
                dtw_sb = pm.tile([128, MB, S], BF)
                nc.sync.dma_start(dtw_sb[:], dp["dtw"][:])
                bpw_sb = pm.tile([128, MB, S], BF)
                nc.sync.dma_start(bpw_sb[:], dp["bpw"][:])
                cpw_sb = pm.tile([128, MB, S], BF)
                nc.sync.dma_start(cpw_sb[:], dp["cpw"][:])
                ew1q = load_ew1q(0)

                # dedicated PSUM pool for premix gate matmuls, opened BEFORE the
                # scan pool so its banks never alias the scan/LN psums — lets the
                # pg prefill run during the serial scan DVE chain.
                from contextlib import ExitStack
                _pgstack = ExitStack()
                psPG = _pgstack.enter_context(
                    tc.tile_pool(name="psPG", bufs=1, space="PSUM"))
                wqs = {}
                pgs = {}

                def emit_pg(m):
                    q, mi = divmod(m, 4)
                    if mi == 0:
                        wq = []
                        for kb in range(KB):
                            wt = pt_pool.tile([128, 512], BF, tag=f"wip{kb}", bufs=2,
                                              name=f"wipg{kb}")
                            nc.gpsimd.dma_start(
                                wt[:], dp["ipw"][kb * 128:(kb + 1) * 128,
                                                 2048 + q * 512:2048 + (q + 1) * 512])
                            wq.append(wt)
                        wqs[q] = wq
                    pg = psPG.tile([128, 512], FP, tag="pg", bufs=4)
                    for kb in range(KB):
                        nc.tensor.matmul(pg[:], wqs[q][kb][:, mi * 128:(mi + 1) * 128],
                                         hT[kb][:, HALO:NH],
                                         start=(kb == 0), stop=(kb == KB - 1))
                    pgs[m] = pg

                # ---- dt/B/C projections + scan ----
                with nc.named_scope("scan"), tc.tile_pool(name="ps3", bufs=1, space="PSUM") as psA:
                    dt_t = pt_pool.tile([S, NH], FP, tag="dt")
                    a_t = pt_pool.tile([S, NH], FP, tag="a")
                    b_t = pt_pool.tile([S, NH], FP, tag="b")
                    c_t = pt_pool.tile([S, NH], FP, tag="c")
                    for n0, nw in ((0, 512), (512, 128)):
                        for wsb, bias_sb, dst, fn in (
                            (dtw_sb, dtb_sb, dt_t, AF.Sigmoid),
                            (cpw_sb, cpb_sb, c_t, AF.Identity),
                        ):
                            pz = psA.tile([S, 512], FP, tag="pz", bufs=2)
                            for kb in range(MB):
                                nc.tensor.matmul(pz[:, 0:nw], wsb[:, kb, :],
                                                 xm[kb][:, n0:n0 + nw],
                                                 start=(kb == 0), stop=(kb == MB - 1))
                            nc.scalar.activation(dst[:, n0:n0 + nw], pz[:, 0:nw], fn,
                                                 bias=bias_sb[:])
                        # b needs dt -> separate pass
                        pz = psA.tile([S, 512], FP, tag="pz", bufs=2)
                        for kb in range(MB):
                            nc.tensor.matmul(pz[:, 0:nw], bpw_sb[:, kb, :],
                                             xm[kb][:, n0:n0 + nw],
                                             start=(kb == 0), stop=(kb == MB - 1))
                        nc.vector.scalar_tensor_tensor(b_t[:, n0:n0 + nw], pz[:, 0:nw],
                                                       bpb_sb[:], dt_t[:, n0:n0 + nw],
                                                       ALU.add, ALU.mult)
                    nc.scalar.activation(a_t[:], dt_t[:], AF.Identity, bias=1.0, scale=-1.0)
                    st_t = pt_pool.tile([S, NH], FP, tag="st")
                    nc.vector.tensor_tensor_scan(st_t[:], a_t[:], b_t[:], 0.0,
                                                 ALU.mult, ALU.add)
                    y_t = pt_pool.tile([S, OWN], FP, tag="dt", name="y_t")
                    nc.vector.tensor_mul(y_t[:], c_t[:, HALO:NH], st_t[:, HALO:NH])

                # prefill gate matmuls: fills the PE while the scan DVE chain runs
                emit_pg(0)
                emit_pg(1)
                emit_pg(2)

                # ---- layernorm over S (transpose - LN - transpose back) ----
                with nc.named_scope("ln"), tc.tile_pool(name="ps4", bufs=1, space="PSUM") as psA:
                    yln = pt_pool.tile([S, OWN], BF, tag="a", name="yln")
                    for i in range(OTB):
                        ptr = psA.tile([128, 128], FP, tag="ptr", bufs=2)
                        nc.tensor.transpose(ptr[:, 0:S], y_t[:, i * 128:(i + 1) * 128],
                                            ident[0:S, 0:S])
                        yT = pt_pool.tile([128, S], FP, tag="yT", bufs=2)
                        nc.vector.tensor_copy(yT[:], ptr[:, 0:S])
                        mu = pt_pool.tile([128, 1], FP, tag="mu", bufs=2)
                        nc.vector.tensor_reduce(mu[:], yT[:], mybir.AxisListType.X, ALU.add)
                        nc.vector.tensor_scalar_mul(mu[:], mu[:], 1.0 / S)
                        xc = pt_pool.tile([128, S], FP, tag="xc", bufs=2)
                        nc.vector.tensor_scalar_sub(xc[:], yT[:], mu[:])
                        scr2 = pt_pool.tile([128, S], FP, tag="scr2", bufs=2)
                        vv = pt_pool.tile([128, 1], FP, tag="vv", bufs=2)
                        nc.scalar.activation(scr2[:], xc[:], AF.Square, accum_out=vv[:])
                        nc.vector.tensor_scalar(vv[:], vv[:], 1.0 / S, 1e-5, ALU.mult, ALU.add)
                        nc.scalar.sqrt(vv[:], vv[:])
                        nc.vector.reciprocal(vv[:], vv[:])
                        nc.vector.tensor_scalar_mul(xc[:], xc[:], vv[:])
                        ptr2 = psA.tile([128, 128], FP, tag="ptr2", bufs=2)
                        nc.tensor.transpose(ptr2[0:S, :], xc[:], ident[:])
                        nc.vector.tensor_copy(yln[:, i * 128:(i + 1) * 128], ptr2[0:S, :])

                s2ib_sb = pm.tile([128, MB], FP)
                nc.sync.dma_start(s2ib_sb[:], dp["s2ib"][:])
                Dp_sb = pm.tile([128, MB], FP)
                nc.sync.dma_start(Dp_sb[:], dp["Dp"][:])
                s2iw_sb = pm.tile([S, INNER], BF)
                nc.sync.dma_start(s2iw_sb[:], dp["s2iw"][:])

                # ---- s2i + gate sigmoid + pre_out assembly ----
                with nc.named_scope("premix"), tc.tile_pool(name="ps5", bufs=1, space="PSUM") as psA:
                    pre = []
                    for m in range(MB):
                        if m + 3 < MB:
                            emit_pg(m + 3)
                        ps = psA.tile([128, 512], FP, tag="ps", bufs=2)
                        nc.tensor.matmul(ps[:], s2iw_sb[:, m * 128:(m + 1) * 128], yln[:],
                                         start=True, stop=True)
                        sg = pt_pool.tile([128, OWN], FP, tag="sg", bufs=2)
                        nc.scalar.activation(sg[:], pgs.pop(m)[:], AF.Sigmoid,
                                             bias=ipb_sb[:, MB + m:MB + m + 1])
                        tmp = pt_pool.tile([128, OWN], FP, tag="tmp", bufs=2)
                        nc.vector.tensor_scalar(tmp[:], xm[m][:, HALO:NH],
                                                Dp_sb[:, m:m + 1], None, ALU.mult)
                        nc.vector.scalar_tensor_tensor(tmp[:], ps[:], s2ib_sb[:, m:m + 1],
                                                       tmp[:], ALU.add, ALU.add)
                        pre_m = pm.tile([128, OWN], BF, tag=f"xm{m}", name=f"pre{m}")
                        nc.vector.tensor_mul(pre_m[:], tmp[:], sg[:])
                        pre.append(pre_m)
                _pgstack.close()

                # ---- out projection + residual + rms2 + h2T + gating ----
                with nc.named_scope("outproj"), tc.tile_pool(name="ps6", bufs=1, space="PSUM") as psA:
                    for nb in range(2):
                        po_t = [psA.tile([128, 512], FP, tag=f"po{t_}", bufs=1,
                                         name=f"po{nb}_{t_}") for t_ in range(OTB)]
                        for kb in range(MB):
                            owt = pt_pool.tile([128, 512], BF, tag="owt", bufs=6)
                            nc.sync.dma_start(
                                owt[:], dp["ow"][kb * 128:(kb + 1) * 128,
                                                 nb * 512:(nb + 1) * 512])
                            for t_ in range(OTB):
                                nc.tensor.matmul(po_t[t_][:],
                                                 pre[kb][:, t_ * 128:(t_ + 1) * 128],
                                                 owt[:], start=(kb == 0), stop=False)
                        for t_ in range(OTB):
                            nc.tensor.matmul(po_t[t_][:], ones1[:],
                                             ob_sb[:, nb * 512:(nb + 1) * 512],
                                             start=False, stop=True)
                            nc.vector.tensor_add(xmid[t_][:, nb * 512:(nb + 1) * 512],
                                                 po_t[t_][:],
                                                 xo[t_][:, nb * 512:(nb + 1) * 512])
                    for tb in range(OTB):
                        # rms2 for this tb
                        scr = pt_pool.tile([128, D], FP, tag="scr", bufs=2)
                        sq = pt_pool.tile([128, 1], FP, tag="sq", bufs=2)
                        nc.scalar.activation(scr[:], xmid[tb][:], AF.Square, accum_out=sq[:])
                        nr = pt_pool.tile([128, 1], FP, tag="nr", bufs=2)
                        nc.vector.tensor_scalar(nr[:], sq[:], 1.0 / D, 1e-6, ALU.mult, ALU.add)
                        nc.scalar.sqrt(nr[:], nr[:])
                        nc.vector.reciprocal(nr[:], nr[:])
                        h2 = pt_pool.tile([128, D], FP, tag="xt", bufs=2, name="h2")
                        nc.vector.tensor_scalar(h2[:], xmid[tb][:], nr[:], None, ALU.mult)
                        pl = psA.tile([128, E], FP, tag="pl", bufs=2)
                        for kb in range(KB):
                            ptr = psA.tile([128, 128], FP, tag="ptr", bufs=2)
                            nc.tensor.transpose(ptr[:], h2[:, kb * 128:(kb + 1) * 128], ident[:])
                            h2T_t = pt_pool.tile([128, 128], FP, tag="h2Tt", bufs=2)
                            nc.vector.tensor_copy(h2T_t[:], ptr[:])
                            nc.vector.tensor_copy(h2T[:, kb, tb * 128:(tb + 1) * 128], h2T_t[:])
                            nc.tensor.matmul(pl[:], h2T_t[:], gw_sb[:, kb, :],
                                             start=(kb == 0), stop=False)
                        nc.tensor.matmul(pl[:], ones1f[:], gb_sb[:], start=False, stop=True)
                        # top-2-of-4 gating -> per-expert combine weights wv[tb]
                        m1 = pt_pool.tile([128, 1], FP, tag="m1", bufs=2)
                        nc.vector.tensor_reduce(m1[:], pl[:], mybir.AxisListType.X, ALU.max)
                        eq1 = pt_pool.tile([128, E], FP, tag="eq1", bufs=2)
                        nc.vector.tensor_scalar(eq1[:], pl[:], m1[:], None, ALU.is_equal)
                        msk = pt_pool.tile([128, E], FP, tag="msk", bufs=2)
                        nc.vector.scalar_tensor_tensor(msk[:], eq1[:], -1e30, pl[:],
                                                       ALU.mult, ALU.add)
                        m2 = pt_pool.tile([128, 1], FP, tag="m2", bufs=2)
                        nc.vector.tensor_reduce(m2[:], msk[:], mybir.AxisListType.X, ALU.max)
                        eq2 = pt_pool.tile([128, E], FP, tag="eq2", bufs=2)
                        nc.vector.tensor_scalar(eq2[:], msk[:], m2[:], None, ALU.is_equal)
                        dd = pt_pool.tile([128, 1], FP, tag="dd", bufs=2)
                        nc.vector.tensor_sub(dd[:], m2[:], m1[:])
                        p2 = pt_pool.tile([128, 1], FP, tag="p2", bufs=2)
                        nc.scalar.activation(p2[:], dd[:], AF.Sigmoid)
                        p1b = pt_pool.tile([128, 1], FP, tag="p1b", bufs=2)
                        nc.scalar.activation(p1b[:], p2[:], AF.Identity, bias=1.0, scale=-1.0)
                        nc.vector.tensor_scalar(wv[tb][:], eq1[:], p1b[:], None, ALU.mult)
                        nc.vector.scalar_tensor_tensor(wv[tb][:], eq2[:], p2[:], wv[tb][:],
                                                       ALU.mult, ALU.add)

            # =======================================================
            # MoE: all 4 experts, own 512 tokens, fully local
            # =======================================================
            with tc.tile_pool(name="moe", bufs=1) as pq:
                # hid as h-pairs for DoubleRow w2: [128, 2, OWN]
                hid = [pq.tile([128, 2, OWN], F8, tag=f"hid{hp}", bufs=1,
                               name=f"hid{hp}") for hp in range(HB // 2)]
                eacc = [pq.tile([128, D], FP, name=f"eacc{t_}", tag=f"eacc{t_}")
                        for t_ in range(OTB)]
                wvs = [pq.tile([128, E], FP, name=f"wvs{t_}", tag=f"wvs{t_}")
                       for t_ in range(OTB)]
                for t_ in range(OTB):
                    nc.vector.tensor_scalar_mul(wvs[t_][:], wv[t_][:], 1.0 / ESCALE)

                HPQ = QW // 128  # h-blocks per eighth = 4
                for e in range(E):
                    with nc.named_scope(f"moe_w1_{e}"), \
                         tc.tile_pool(name=f"psW1_{e}", bufs=1, space="PSUM") as psA:
                        for h in range(HB):
                            g = e * NQ + h // HPQ
                            if h % HPQ == 0:
                                cur = ew1q if g == 0 else nxt
                                if g + 1 < E * NQ:
                                    nxt = load_ew1q(g + 1)
                            ph = psA.tile([128, 512], FP, tag="ph", bufs=3)
                            hc = (h % HPQ) * 128
                            for kb in range(KB):
                                nc.tensor.matmul(ph[:], cur[:, kb, hc:hc + 128],
                                                 h2T[:, kb, :],
                                                 start=(kb == 0), stop=(kb == KB - 1))
                            nc.scalar.activation(hid[h // 2][:, h % 2, :], ph[:], AF.Gelu,
                                                 bias=eb1_sb[:, e * HB + h:e * HB + h + 1])
                    with nc.named_scope(f"moe_w2_{e}"), \
                         tc.tile_pool(name=f"psW2_{e}", bufs=1, space="PSUM") as psB:
                        peo = [[psB.tile([128, 512], FP, tag=f"peo{nb}_{t_}", bufs=1,
                                         name=f"peo{nb}_{t_}") for t_ in range(OTB)]
                               for nb in range(2)]
                        for hp in range(HB // 2):
                            ew2t = pq.tile([128, 2, D], F8, tag="ew2t", bufs=6)
                            nc.gpsimd.dma_start(
                                ew2t[:],
                                dp["ew2"][(e * HB // 2 + hp) * 128:
                                          (e * HB // 2 + hp + 1) * 128, :])
                            for nb in range(2):
                                for t_ in range(OTB):
                                    nc.tensor.matmul(
                                        peo[nb][t_][:],
                                        hid[hp][:, :, t_ * 128:(t_ + 1) * 128],
                                        ew2t[:, :, nb * 512:(nb + 1) * 512],
                                        start=(hp == 0),
                                        stop=(eb2_zero and hp == HB // 2 - 1),
                                        perf_mode=DR)
                        for nb in range(2):
                            for t_ in range(OTB):
                                if not eb2_zero:
                                    nc.tensor.matmul(
                                        peo[nb][t_][:], ones18[:],
                                        eb2_sb[:, e * D + nb * 512: e * D + (nb + 1) * 512],
                                        start=False, stop=True)
                                nc.vector.scalar_tensor_tensor(
                                    eacc[t_][:, nb * 512:(nb + 1) * 512],
                                    peo[nb][t_][:], wvs[t_][:, e:e + 1],
                                    (xmid if e == 0 else eacc)[t_][:, nb * 512:(nb + 1) * 512],
                                    ALU.mult, ALU.add)
                                if e == E - 1:
                                    eng = nc.sync if (nb * OTB + t_) % 2 == 0 else nc.scalar
                                    eng.dma_start(
                                        out_d[t_ * 128:(t_ + 1) * 128,
                                              nb * 512:(nb + 1) * 512],
                                        eacc[t_][:, nb * 512:(nb + 1) * 512])

    nc.compile()
    return nc


def host_prep(inputs):
    """Build the 8 per-core input maps from full inputs."""
    import ml_dtypes
    f32 = np.float32
    bf16 = ml_dtypes.bfloat16
    x = np.ascontiguousarray(np.asarray(inputs["x"], f32).reshape(B * T, D))
    n1 = np.asarray(inputs["norm1_w"], f32)
    n2 = np.asarray(inputs["norm2_w"], f32)

    def pcol(v, nb):  # [nb*128] -> [128, nb], col b = block b
        return np.ascontiguousarray(np.asarray(v, f32).reshape(nb, 128).T)

    ipw = (np.asarray(inputs["in_proj_w"], f32) * n1[:, None]).astype(bf16)
    gwf = np.asarray(inputs["gate_w"], f32) * n2[:, None]          # [D, E]
    gw = np.ascontiguousarray(gwf.reshape(KB, 128, E).swapaxes(0, 1))  # [128,KB,E]
    fp8 = ml_dtypes.float8_e4m3fn
    ew1f = np.asarray(inputs["e_w1"], f32) * n2[None, :, None]     # [E,D,HID]
    ew1 = np.ascontiguousarray(ew1f.reshape(E * D, HID)).astype(bf16)
    # DoubleRow pair layout: row (e,hp,p) holds [two, d]
    ew2f = np.asarray(inputs["e_w2"], f32) * ESCALE                # [E,HID,D]
    ew2 = np.ascontiguousarray(
        ew2f.reshape(E, HB // 2, 2, 128, D).swapaxes(2, 3)
        .reshape(E * HID // 2, 2 * D)).astype(fp8)
    eb1f = np.asarray(inputs["e_b1"], f32)                         # [E, HID]
    # eb1[p, e*HB+h] = e_b1[e, h*128+p]
    eb1 = np.ascontiguousarray(
        eb1f.reshape(E, HB, 128).transpose(2, 0, 1).reshape(128, E * HB))
    eb2 = (np.asarray(inputs["e_b2"], f32).reshape(1, E * D) * ESCALE).astype(fp8)

    def kw(v):  # [INNER, S] -> [128, MB, S]
        return np.ascontiguousarray(
            np.asarray(v, f32).reshape(MB, 128, S).swapaxes(0, 1)).astype(bf16)

    cwf = np.asarray(inputs["conv_w"], f32)[:, 0, :]               # [INNER, 3]
    cw = np.ascontiguousarray(cwf.reshape(MB, 128, 3).swapaxes(0, 1))

    shared = {
        "ipw": ipw,
        "ipb": pcol(inputs["in_proj_b"], 2 * INNER // 128),
        "cw": cw, "cb": pcol(inputs["conv_b"], MB),
        "dtw": kw(inputs["dt_w"]), "dtb": np.asarray(inputs["dt_b"], f32).reshape(S, 1),
        "bpw": kw(inputs["bp_w"]), "bpb": np.asarray(inputs["bp_b"], f32).reshape(S, 1),
        "cpw": kw(inputs["cp_w"]), "cpb": np.asarray(inputs["cp_b"], f32).reshape(S, 1),
        "s2iw": np.asarray(inputs["s2i_w"], f32).astype(bf16),
        "s2ib": pcol(inputs["s2i_b"], MB),
        "Dp": pcol(inputs["D_param"], MB),
        "ow": np.asarray(inputs["out_w"], f32).astype(bf16),
        "ob": np.asarray(inputs["out_b"], f32).reshape(1, D).astype(bf16),
        "gw": gw, "gb": np.asarray(inputs["gate_b"], f32).reshape(1, E),
        "ew1": ew1, "eb1": eb1, "ew2": ew2, "eb2": eb2,
        "ident": np.eye(128, dtype=f32),
        "identb": np.eye(128, dtype=f32).astype(bf16),
        "ones1": np.ones((1, 128), f32).astype(bf16),
        "ones18": np.ones((1, 128), f32).astype(fp8),
        "ones1f": np.ones((1, 128), f32),
    }
    in_maps = []
    for c in range(N_CORES):
        g0 = c * OWN
        if g0 % T == 0:
            x_sh = np.concatenate([np.zeros((HALO, D), f32), x[g0:g0 + OWN]])
        else:
            x_sh = x[g0 - HALO:g0 + OWN]
        m = dict(shared)
        m["x_sh"] = np.ascontiguousarray(x_sh)
        in_maps.append(m)
    return in_maps


def unshard_out(results):
    """results: list of 8 dicts with 'out' [OWN, D]; core c holds global
    tokens [c*512, (c+1)*512)."""
    full = np.concatenate([results[c]["out"] for c in range(N_CORES)], axis=0)
    return full.reshape(B, T, D)


_NC_CACHE = {}


def _get_nc(eb2_zero=True):
    key = ("nc", eb2_zero)
    if key not in _NC_CACHE:
        _NC_CACHE[key] = build(eb2_zero=eb2_zero)
    return _NC_CACHE[key]


def kernel(**inputs) -> np.ndarray:
    """Full-input entry point: shards across 8 NeuronCores, runs the Bass
    kernel SPMD, reassembles the full [2, 2048, 1024] output."""
    import sys, types
    try:  # NTFF profile hook shim (missing antenv.axon_hooks in this image)
        import antenv.axon_hooks  # noqa: F401
    except ImportError:
        try:
            import antenv
            from trn_agent_boot.trn_boot import _ntff_profile_via_ctypes
            mod = types.ModuleType("antenv.axon_hooks")
            try:
                _hook = _ntff_profile_via_ctypes("/opt/axon/libaxon_pjrt.so")
            except Exception:
                _hook = None
            mod.get_axon_ntff_profile_hook = lambda: _hook
            mod.set_axon_ntff_profile_hook = lambda h: None
            sys.modules["antenv.axon_hooks"] = mod
            antenv.axon_hooks = mod
        except Exception:
            pass
    from concourse.bass_utils import run_bass_kernel_spmd

    nc = _get_nc(eb2_zero=not np.any(np.asarray(inputs["e_b2"])))
    in_maps = host_prep(inputs)
    res = run_bass_kernel_spmd(nc, in_maps, core_ids=list(range(N_CORES)))
    out = unshard_out(res.results)
    return out.astype(np.float32)
